# revision 23
# baseline (speedup 1.0000x reference)
"""Trainium2 Bass kernel for nn_NodeInference (2-layer GAT + cosine head).

v4 design (SPMD over 8 cores, dst-node sharding, hybrid gather/dense):
  Host globally re-assigns nodes to (core, block) bins, balancing per-block
  in-degree.  Both GAT layers share ONE edge-slot layout (chunks of 128
  edges per dst block, split lo/hi by global table row for int16 gather
  indices), so the per-edge one-hot matrices are built once on the host and
  shipped:
     std  [e-transposed]  st[d,(j,e)]  = (dloc[j,e]==d)   (ad lookup lhsT)
     sald                 sall[e,(j,d)] = (dloc[j,e]==d)  (scatter base)
  This removes the per-block PE broadcast + DVE is_equal chains of v2.

  P1  sharded dense: each core computes h1aug only for its OWN 6272 nodes
      -> cc1_in rows [h1|1|h2|1|as f32 x2|ad f32 x2] (768B)
  AG1 AllGather cc1_in -> cc1_out (global h1 table), in 2 pieces
  P2  edge phase 1 per dst block: chunks are HYBRID:
      - gather chunks: dma_gather rows from cc1_out (GpSimd)
      - dense chunks:  gt[:,j] = x[src_e] @ W1aug on the PE (x[src_e] is a
        host input, shipped pre-arranged per edge slot in xsTi) -- trades
        GpSimd descriptor-generation time for PE time to balance engines
      - a_d per edge = st_j^T @ adwin;  w_e = exp(min(lrelu(a_s+a_d,.2),30))
      - scatter: bp += (sall*w_h)_j^T @ rows_j  (rows carry literal 1.0 so
        the same matmul accumulates the softmax denominator)
      - epilogue -> out1T; h2aug rows -> cc2_in
      Blocks are software-pipelined (stage A: dma/dense/gather/ad of block
      b+1 emitted before stage B: ew/swh/scatter/epilogue of block b) to
      avoid in-order PE stalls.
  AG2 AllGather cc2_in -> cc2_out in 2 pieces, piece 0 issued mid-phase
  P4  edge phase 2: all chunks gathered from cc2_out (content is
      device-computed, so the host x-trick cannot apply)
  P5  head: cos sim vs mu -> outT [8, SHARD_CAP], interleaved into P4
Host scatters per-core outT into the full output via the assignment map.
"""

import sys
from dataclasses import dataclass, field
from contextlib import ExitStack

if "/opt/trn_rl_repo" not in sys.path:
    sys.path.insert(0, "/opt/trn_rl_repo")

import numpy as np

import concourse.bacc as bacc
import concourse.bass as bass
import concourse.mybir as mybir
import concourse.tile as tile
from concourse.bass import AP

P = 128
IN = 256          # input feature dim
H1 = 2            # layer-1 heads
HID = 256         # layer-1 output dim (2*128, concat)
OUT = 256         # layer-2 output dim
KH, MD = 8, 128   # cosine head shape
ROWW = 384        # fp16 cols per packed table row (768B)
HALF = 32768      # int16 table-half split
DENSE_LO = 5      # layer-1 lo chunks computed on the PE instead of gathered
DENSE_HI = 4      # layer-1 hi chunks computed on the PE
AF = mybir.ActivationFunctionType
ALU = mybir.AluOpType
DT = mybir.dt


@dataclass
class CFG:
    N: int
    W: int              # world size
    NBLK: int           # dst blocks (128 dsts) per core
    CPL: int            # lo-half chunks per block
    CPH: int
    idxmaps: object = field(default=None, repr=False)

    @property
    def SHARD_CAP(self):
        return self.NBLK * P

    @property
    def CPB(self):
        return self.CPL + self.CPH

    @property
    def DCL(self):
        return min(DENSE_LO, self.CPL)

    @property
    def DCH(self):
        return min(DENSE_HI, self.CPH)

    @property
    def DCT(self):
        return self.DCL + self.DCH

    def dsched(self, blk):
        """(dense_lo, dense_hi) for a block; the first blocks are fully
        dense so they have no AllGather dependency and run during AG1."""
        if blk < 6:
            return (self.CPL, self.CPH)
        if blk < 10:
            return (min(8, self.CPL), min(4, self.CPH))
        return (self.DCL, self.DCH)

    @property
    def xoffs(self):
        offs, t = [], 0
        for b in range(self.NBLK):
            dl, dh = self.dsched(b)
            offs.append(t)
            t += dl + dh
        offs.append(t)
        return offs

    @property
    def PB0(self):       # blocks in AllGather piece 0 (int16 row limit)
        return min(self.NBLK, HALF // (self.W * P))


def build_program(cfg: CFG):
    nc = bacc.Bacc("TRN2", target_bir_lowering=False, debug=False)
    W, NBLK = cfg.W, cfg.NBLK
    AUG1, AUG2 = IN + 4, IN + 2
    f16, bf16, f32 = DT.float16, DT.bfloat16, DT.float32
    i16 = DT.int16
    CPB, CPL, CPH = cfg.CPB, cfg.CPL, cfg.CPH
    DCL, DCH = cfg.DCL, cfg.DCH
    PB0, PB1 = cfg.PB0, cfg.NBLK - cfg.PB0
    CAP = cfg.SHARD_CAP

    with tile.TileContext(nc) as tc, ExitStack() as stack:
        dram = stack.enter_context(
            tc.tile_pool(name="dram", bufs=1, space="DRAM"))

        def din(name, shape, dtype):
            return dram.tile(shape, dtype, kind="ExternalInput", name=name,
                             uniquify=False)

        xoTi = din("xoTi", [P, NBLK, 2, P], f16)
        XOFF = cfg.xoffs
        DCTMAX = max(sum(cfg.dsched(b)) for b in range(NBLK))
        xsTi = din("xsTi", [P, XOFF[NBLK], 2, P], f16)
        isd = din("isd", [P, NBLK * CPB * 8], i16)
        std = din("std", [P, NBLK * CPB * P], bf16)
        sald = din("sald", [P, NBLK * CPB * P], bf16)
        w1s = din("w1s", [P, 2, AUG1], f16)
        w2s = din("w2s", [P, 2, AUG2], f16)
        gsd = din("gs", [P, 2, KH * P], f16)
        gmud = din("gmu", [P, 2, KH], f16)        # sum_m g[f,km]*mu[k,m]
        ond = din("onesd", [P, KH * KH], f16)     # block-diag ones
        cmu = din("cmu", [KH, 1], f32)
        b1d = din("b1b", [P, HID], f32)
        b2d = din("b2b", [P, OUT], f32)
        idn = din("ident", [P, P], f32)
        idb = din("identb", [P, P], bf16)
        outT = dram.tile([KH, CAP], f32, kind="ExternalOutput",
                         name="outT", uniquify=False)

        shsp = "Shared" if W > 1 else "Local"
        cc1_in = dram.tile([CAP, ROWW], f16, name="cc1_in")
        cc1_p0 = dram.tile([W * PB0 * P, ROWW], f16, name="cc1_p0",
                           addr_space=shsp)
        cc1_p1 = dram.tile([W * PB1 * P, ROWW], f16, name="cc1_p1",
                           addr_space=shsp)
        cc2_in = dram.tile([CAP, ROWW], f16, name="cc2_in")
        cc2_p0 = dram.tile([W * PB0 * P, ROWW], f16, name="cc2_p0",
                           addr_space=shsp)
        cc2_p1 = dram.tile([W * PB1 * P, ROWW], f16, name="cc2_p1",
                           addr_space=shsp)

        consts = stack.enter_context(tc.tile_pool(name="consts", bufs=1))
        w1_sb = consts.tile([P, 2, AUG1], f16)
        w2_sb = consts.tile([P, 2, AUG2], f16)
        g_sb = consts.tile([P, 2, KH * P], f16)
        gmu_sb = consts.tile([P, 2, KH], f16)
        on_sb = consts.tile([P, KH * KH], f16)
        cmu_sb = consts.tile([KH, 1], f32)
        b1_sb = consts.tile([P, HID], f32)
        b2_sb = consts.tile([P, OUT], f32)
        ident_sb = consts.tile([P, P], f32)
        identb_sb = consts.tile([P, P], bf16)
        out1T_sb = consts.tile([P, 2, CAP], f16)
        h2fT_sb = consts.tile([P, 2, CAP], f16)

        for dst, src in [(w1_sb, w1s), (w2_sb, w2s), (g_sb, gsd),
                         (gmu_sb, gmud), (on_sb, ond), (cmu_sb, cmu),
                         (b1_sb, b1d), (b2_sb, b2d),
                         (ident_sb, idn), (identb_sb, idb)]:
            nc.sync.dma_start(dst[:], src[:])

        def ag_piece(cin, cout_p, pc):
            """AllGather piece pc (0/1) of cin into its own Shared tensor."""
            lr = (0, PB0 * P) if pc == 0 else (PB0 * P, CAP)
            nc.gpsimd.collective_compute(
                "AllGather", ALU.bypass,
                replica_groups=[list(range(W))],
                ins=[cin[lr[0]:lr[1], :]], outs=[cout_p[:, :]])

        # ================= P1: sharded dense (own nodes only) ===============
        with tc.tile_pool(name="p1x", bufs=3) as p1x, \
             tc.tile_pool(name="p1ps", bufs=3, space="PSUM") as p1ps, \
             tc.tile_pool(name="p1row", bufs=3) as p1row:
            for blk in range(NBLK):
                xo = p1x.tile([P, 2, P], f16, tag="xo")
                nc.sync.dma_start(xo[:], xoTi[:, blk, :, :])
                ps = p1ps.tile([P, AUG1], f32, tag="ps")
                for k in range(2):
                    nc.tensor.matmul(ps[:], lhsT=xo[:, k, :],
                                     rhs=w1_sb[:, k, :],
                                     start=(k == 0), stop=(k == 1))
                # row: [h1 0:128 | 1.0 | h2 129:257 | 1.0 | as f32 x2 | ad..]
                row = p1row.tile([P, 272], f16, tag="row")
                rf32 = row[:].bitcast(f32)
                nc.scalar.activation(row[:, 0:P], ps[:, 0:P], AF.Copy)
                nc.scalar.activation(row[:, P + 1:IN + 1],
                                     ps[:, P:IN], AF.Copy)
                nc.vector.tensor_copy(rf32[:, 129:133], ps[:, IN:IN + 4])
                nc.vector.memset(row[:, P:P + 1], 1.0)
                nc.vector.memset(row[:, IN + 1:IN + 2], 1.0)
                nc.sync.dma_start(
                    cc1_in[blk * P:(blk + 1) * P, 0:IN + 10],
                    row[:, 0:IN + 10])
                if blk == PB0 - 1:
                    ag_piece(cc1_in, cc1_p0, 0)
            ag_piece(cc1_in, cc1_p1, 1)

        # ================= edge phases ======================================
        # partial L2 sums (lo sub-phase) parked per block in DRAM (f32 --
        # the exp() weights overflow f16): rows [f 0:256 | sumw]
        part_d = dram.tile([CAP, IN + 1], f32, name="part_d")

        def edge_phase(layer, part=None, post_block=None):
            if layer == 1:
                nhead = 2
                b_sb, out_t, lrelu_out = b1_sb, out1T_sb, True
                as_off, ad_off = 129, 131   # f32 col offsets in table rows
                cin = cc1_in
                c0c, c1c = 0, CPB
                with_self, do_epi = True, True
                # chunk roles are per-block (cfg.dsched): lo/hi tails are
                # dense-computed, the rest gathered
                g_rngs = d_rngs = True   # per-block, built in stage_a
            else:
                nhead = 1
                b_sb, out_t, lrelu_out = b2_sb, h2fT_sb, False
                as_off, ad_off = 129, 130
                cin = cc2_in
                d_rngs = []
                SPC = min(CPL, 8)   # pass-A chunk count (one gather call)
                if part == 'lo':
                    c0c, c1c = 0, SPC
                    with_self, do_epi = False, False
                    g_rngs = [(0, SPC, cc2_p0)]
                else:
                    c0c, c1c = SPC, CPB
                    with_self, do_epi = True, True
                    g_rngs = [(SPC, CPL, cc2_p0), (CPL, CPB, cc2_p1)]
            NCF = c1c - c0c
            sfx = f"{layer}{part or ''}"

            estack = ExitStack()
            pi = estack.enter_context(
                tc.tile_pool(name=f"idx{sfx}", bufs=4))
            pso = estack.enter_context(
                tc.tile_pool(name=f"soh{sfx}", bufs=4))
            pg = estack.enter_context(
                tc.tile_pool(name=f"gath{sfx}", bufs=4))
            pw = estack.enter_context(
                tc.tile_pool(name=f"win{sfx}", bufs=4))
            px = (estack.enter_context(tc.tile_pool(name="dsx", bufs=2))
                  if d_rngs else None)
            pps = (estack.enter_context(
                tc.tile_pool(name="dsps", bufs=2, space="PSUM"))
                if d_rngs else None)
            pad_ = estack.enter_context(
                tc.tile_pool(name=f"adps{sfx}", bufs=2, space="PSUM"))
            pe_ = estack.enter_context(tc.tile_pool(name=f"ew{sfx}", bufs=4))
            pm = estack.enter_context(tc.tile_pool(name=f"sw{sfx}", bufs=2))
            pp = estack.enter_context(
                tc.tile_pool(name=f"bps{sfx}", bufs=2, space="PSUM"))
            pt = (estack.enter_context(
                tc.tile_pool(name=f"tps{sfx}", bufs=1, space="PSUM"))
                if do_epi else None)
            po = estack.enter_context(tc.tile_pool(name=f"epi{sfx}", bufs=2))
            ph = (estack.enter_context(
                tc.tile_pool(name=f"h2ps{sfx}", bufs=1, space="PSUM"))
                if layer == 1 else None)

            NSW = NCF + (1 if with_self else 0)
            state = {}

            def stage_a(blk):
                cb8 = blk * CPB * 8
                cbp = blk * CPB * P
                isdt = pi.tile([P, NCF * 8], i16, tag="isd")
                nc.sync.dma_start(isdt[:], isd[:, cb8 + c0c * 8:
                                                cb8 + c1c * 8])
                st = pso.tile([P, NCF * P], bf16, tag="st")
                nc.sync.dma_start(st[:], std[:, cbp + c0c * P:
                                             cbp + c1c * P])
                sall = pso.tile([P, NCF * P], bf16, tag="sall")
                nc.sync.dma_start(sall[:], sald[:, cbp + c0c * P:
                                                cbp + c1c * P])
                win = pw.tile([P, ROWW], f16, tag="win")
                nc.sync.dma_start(win[:], cin[blk * P:(blk + 1) * P, :])
                winf = win[:].bitcast(f32)

                gt = pg.tile([P, NCF, ROWW], f16, tag="gt")
                gtf = gt[:].bitcast(f32)
                if layer == 1:
                    dlo, dhi = cfg.dsched(blk)
                    g_rb = [(0, CPL - dlo, cc1_p0), (CPL, CPB - dhi, cc1_p1)]
                    d_rb = [(CPL - dlo, CPL, 0), (CPB - dhi, CPB, dlo)]
                else:
                    g_rb, d_rb = g_rngs, []
                # gather chunks (chunk j maps to local slot j - c0c)
                MXC = 8
                for r0, r1, tab in g_rb:
                    if r1 <= r0:
                        continue
                    for c0 in range(r0, r1, MXC):
                        c1 = min(c0 + MXC, r1)
                        nc.gpsimd.dma_gather(
                            gt[:, c0 - c0c:c1 - c0c, :], tab[:, :],
                            isdt[:, (c0 - c0c) * 8:(c1 - c0c) * 8],
                            (c1 - c0) * P, (c1 - c0) * P, ROWW)
                # dense chunks (layer 1 only)
                if d_rb:
                    dct_b = sum(cfg.dsched(blk))
                    xt = px.tile([P, DCTMAX, 2, P], f16, tag="xt")
                    xo = XOFF[blk]
                    nc.sync.dma_start(xt[:, 0:dct_b, :, :],
                                      xsTi[:, xo:xo + dct_b, :, :])
                    for r0, r1, xb in d_rb:
                        for j in range(r0, r1):
                            ps = pps.tile([P, AUG1], f32, tag="ps")
                            xi = xb + (j - r0)
                            for k in range(2):
                                nc.tensor.matmul(ps[:], lhsT=xt[:, xi, k, :],
                                                 rhs=w1_sb[:, k, :],
                                                 start=(k == 0), stop=(k == 1))
                            nc.scalar.activation(gt[:, j, 0:P], ps[:, 0:P],
                                                 AF.Copy)
                            nc.scalar.activation(gt[:, j, P + 1:IN + 1],
                                                 ps[:, P:IN], AF.Copy)
                            nc.vector.tensor_copy(gtf[:, j, 129:131],
                                                  ps[:, IN:IN + 2])
                        nc.vector.memset(gt[:, r0:r1, P:P + 1], 1.0)
                        nc.vector.memset(gt[:, r0:r1, IN + 1:IN + 2], 1.0)

                # a_d per edge: ad_ps[e, (j,h)] = st_j^T @ adwin
                adw16 = pe_.tile([P, nhead], f16, tag="adw16")
                nc.vector.tensor_copy(adw16[:],
                                      winf[:, ad_off:ad_off + nhead])
                ad_ps = pad_.tile([P, NCF * nhead], f32, tag="adps")
                for j in range(NCF):
                    nc.tensor.matmul(ad_ps[:, j * nhead:(j + 1) * nhead],
                                     lhsT=st[:, j * P:(j + 1) * P],
                                     rhs=adw16[:], start=(j == 0),
                                     stop=(j == NCF - 1))
                pl = None
                if layer == 2 and do_epi:
                    pl = po.tile([P, IN + 1], f32, tag="pl")
                    nc.sync.dma_start(pl[:],
                                      part_d[blk * P:(blk + 1) * P, :])
                state[blk] = (gt, gtf, win, winf, sall, ad_ps, pl)

            def stage_b(blk):
                gt, gtf, win, winf, sall, ad_ps, pl = state.pop(blk)
                # edge weights w = exp(lrelu(a_s + a_d, .2))
                ew = pe_.tile([P, NCF * nhead], f32, tag="ew")
                wv = pe_.tile([P, NCF * nhead], bf16, tag="wv")
                as_v = gtf[:, :, as_off:as_off + nhead]
                nc.vector.tensor_tensor(ew[:], as_v, ad_ps[:], op=ALU.add)
                ewl = pe_.tile([P, NCF * nhead], f32, tag="ewl")
                nc.vector.scalar_tensor_tensor(
                    out=ewl[:], in0=ew[:], scalar=0.2, in1=ew[:],
                    op0=ALU.mult, op1=ALU.max)
                nc.scalar.activation(wv[:], ewl[:], AF.Exp)

                if with_self:
                    # self-loop weights from the window rows
                    ws = pe_.tile([P, nhead], f32, tag="ws")
                    nc.vector.tensor_tensor(
                        ws[:], winf[:, as_off:as_off + nhead],
                        winf[:, ad_off:ad_off + nhead], op=ALU.add)
                    wt = pe_.tile([P, nhead], f32, tag="wt")
                    nc.vector.scalar_tensor_tensor(
                        out=wt[:], in0=ws[:], scalar=0.2, in1=ws[:],
                        op0=ALU.mult, op1=ALU.max)
                    nc.scalar.activation(wt[:], wt[:], AF.Exp)

                # weighted one-hot scatter: sw_h = sall * w_h (stride-0 bcast)
                swh = []
                for h in range(nhead):
                    wvh = wv[:, h:h + 1]
                    wview = AP(wvh.tensor, wvh.offset,
                               [wvh.ap[0], [nhead, NCF], [0, P]])
                    swt = pm.tile([P, NCF * P], bf16, tag=f"swh{h}")
                    nc.vector.tensor_tensor(swt[:], sall[:], wview,
                                            op=ALU.mult)
                    swh.append(swt)

                # single psum accumulation group across all j/h matmuls:
                # start only on the first, stop only on the very last.  Table
                # rows carry a literal 1.0 after each head's features, so one
                # matmul accumulates both the weighted feature sum and the
                # softmax denominator:
                # bp layout [f1 0:128 | sumw1 | f2 129:257 | sumw2] (2 heads)
                # or [f 0:256 | sumw] (1 head).
                HB = P + 1 if nhead == 2 else IN + 1
                bp = pp.tile([P, nhead * HB], f32, tag="bp")
                for j in range(NSW):
                    last = (j == NSW - 1)
                    selfc = with_self and (j == NCF)
                    for h in range(nhead):
                        if selfc:
                            sw = pm.tile([P, P], bf16, tag="sw")
                            nc.vector.tensor_scalar(
                                out=sw[:], in0=identb_sb[:],
                                scalar1=wt[:, h:h + 1],
                                scalar2=None, op0=ALU.mult)
                            lhsT = sw[:]
                        else:
                            lhsT = swh[h][:, j * P:(j + 1) * P]
                        c0, c1 = h * HB, (h + 1) * HB
                        rhs = win[:, c0:c1] if selfc else gt[:, j, c0:c1]
                        nc.tensor.matmul(bp[:, c0:c1], lhsT=lhsT,
                                         rhs=rhs,
                                         start=(j == 0 and h == 0),
                                         stop=(last and h == nhead - 1))

                if not do_epi:
                    # L2 lo sub-phase: park the partial sums, finish later
                    pf = po.tile([P, IN + 1], f32, tag="pf")
                    nc.scalar.activation(pf[:], bp[:], AF.Copy)
                    nc.sync.dma_start(part_d[blk * P:(blk + 1) * P, :],
                                      pf[:])
                    if post_block is not None:
                        post_block(blk)
                    return

                # ---- block epilogue
                if layer == 2:
                    tb = po.tile([P, IN + 1], f32, tag="tb")
                    nc.vector.tensor_tensor(tb[:], bp[:], pl[:],
                                            op=ALU.add)
                    bpv = tb
                else:
                    bpv = bp
                rec = po.tile([P, nhead], f32, tag="rec")
                for h in range(nhead):
                    nc.vector.reciprocal(rec[:, h:h + 1],
                                         bpv[:, (h + 1) * HB - 1:
                                             (h + 1) * HB])
                ti = po.tile([P, IN], f32, tag="ti")
                if nhead == 2:
                    nc.scalar.activation(ti[:, 0:P], bpv[:, 0:P], AF.Copy,
                                         scale=rec[:, 0:1])
                    nc.scalar.activation(ti[:, P:IN], bpv[:, HB:HB + P],
                                         AF.Copy, scale=rec[:, 1:2])
                else:
                    nc.scalar.activation(ti[:], bpv[:, 0:IN], AF.Copy,
                                         scale=rec[:, 0:1])
                nc.vector.tensor_tensor(ti[:], ti[:], b_sb[:], op=ALU.add)
                if lrelu_out:
                    # lrelu(x) = 0.01x + relu(0.99x)
                    tr = po.tile([P, IN], f32, tag="tr")
                    nc.scalar.activation(tr[:], ti[:], AF.Relu, scale=0.99)
                    nc.vector.scalar_tensor_tensor(
                        out=ti[:], in0=ti[:], scalar=0.01, in1=tr[:],
                        op0=ALU.mult, op1=ALU.add)
                for k in range(2):
                    tp = pt.tile([P, P], f32, tag="tp")
                    nc.tensor.transpose(tp[:], ti[:, k * P:(k + 1) * P],
                                        ident_sb[:])
                    nc.scalar.activation(out_t[:, k, blk * P:(blk + 1) * P],
                                         tp[:], AF.Copy)

                if layer == 1:
                    hp = ph.tile([P, AUG2], f32, tag="hp")
                    for k in range(2):
                        nc.tensor.matmul(
                            hp[:],
                            lhsT=out1T_sb[:, k, blk * P:(blk + 1) * P],
                            rhs=w2_sb[:, k, :], start=(k == 0), stop=(k == 1))
                    # L2 row: [h 0:256 | 1.0 | a_s f32 @129 | a_d f32 @130]
                    row2 = po.tile([P, ROWW], f16, tag="row2")
                    nc.scalar.activation(row2[:, 0:OUT], hp[:, 0:OUT], AF.Copy)
                    nc.vector.memset(row2[:, OUT:OUT + 2], 1.0)
                    r2f = row2[:].bitcast(f32)
                    nc.vector.tensor_copy(r2f[:, 129:131],
                                          hp[:, OUT:OUT + 2])
                    nc.sync.dma_start(
                        cc2_in[blk * P:(blk + 1) * P, 0:OUT + 6],
                        row2[:, 0:OUT + 6])

                if post_block is not None:
                    post_block(blk)

            for b in range(NBLK + 2):
                if b >= 2:
                    stage_b(b - 2)
                if b < NBLK:
                    stage_a(b)

            estack.close()

        AGI0 = min(NBLK - 2, PB0 + 6)

        def _post1(blk):
            if blk == AGI0:
                ag_piece(cc2_in, cc2_p0, 0)
            elif blk == NBLK - 1:
                ag_piece(cc2_in, cc2_p1, 1)

        edge_phase(1, post_block=_post1)
        edge_phase(2, part='lo')

        # ====== P5 head, interleaved into edge phase 2 ======================
        hstack = ExitStack()
        hps = hstack.enter_context(
            tc.tile_pool(name="hps", bufs=1, space="PSUM"))
        hsb = hstack.enter_context(tc.tile_pool(name="hsb", bufs=2))
        sps = hstack.enter_context(
            tc.tile_pool(name="sps", bufs=1, space="PSUM"))
        hepi = hstack.enter_context(tc.tile_pool(name="hepi", bufs=2))

        NTL = []
        _st = 0
        while _st < CAP:
            _w = min(512 if _st + 1024 <= CAP else 256, CAP - _st)
            NTL.append((_st, _w))
            _st += _w

        def head_slice(st, wdt):
            nump = sps.tile([KH, 512], f32, tag="nump")
            nrmp = sps.tile([KH, 512], f32, tag="nrmp")
            # numerator: num[k,n] = sum_f Gmu[f,k] * h2[f,n]
            for f in range(2):
                nc.tensor.matmul(nump[:, 0:wdt],
                                 lhsT=gmu_sb[:, f, :],
                                 rhs=h2fT_sb[:, f, st:st + wdt],
                                 start=(f == 0), stop=(f == 1))
            for k in range(KH):
                hp = hps.tile([P, 512], f32, tag="hp")
                for f in range(2):
                    nc.tensor.matmul(hp[:, 0:wdt],
                                     lhsT=g_sb[:, f, k * P:(k + 1) * P],
                                     rhs=h2fT_sb[:, f, st:st + wdt],
                                     start=(f == 0), stop=(f == 1))
                sq16 = hsb.tile([P, 512], f16, tag="sq16")
                nc.scalar.activation(sq16[:, 0:wdt], hp[:, 0:wdt], AF.Square)
                nc.tensor.matmul(nrmp[:, 0:wdt],
                                 lhsT=on_sb[:, k * KH:(k + 1) * KH],
                                 rhs=sq16[:, 0:wdt], start=(k == 0),
                                 stop=(k == KH - 1))
            sq = hepi.tile([KH, 512], f32, tag="sqr")
            # sqrt(x) = exp(0.5*ln(x)) -- keeps ACT on the ln/exp table set
            nc.scalar.activation(sq[:, 0:wdt], nrmp[:, 0:wdt], AF.Ln)
            nc.scalar.activation(sq[:, 0:wdt], sq[:, 0:wdt], AF.Exp,
                                 scale=0.5)
            nc.vector.tensor_scalar(out=sq[:, 0:wdt], in0=sq[:, 0:wdt],
                                    scalar1=cmu_sb[:, 0:1], scalar2=1e-8,
                                    op0=ALU.mult, op1=ALU.max)
            nc.vector.reciprocal(sq[:, 0:wdt], sq[:, 0:wdt])
            res = hepi.tile([KH, 512], f32, tag="res")
            nc.vector.tensor_tensor(res[:, 0:wdt], nump[:, 0:wdt],
                                    sq[:, 0:wdt], op=ALU.mult)
            nc.sync.dma_start(outT[:, st:st + wdt], res[:, 0:wdt])

        _emitted = [0]

        def _post2(blk):
            done = (blk + 1) * P
            while _emitted[0] < len(NTL):
                st, wdt = NTL[_emitted[0]]
                if st + wdt > done:
                    break
                head_slice(st, wdt)
                _emitted[0] += 1

        edge_phase(2, part='hi', post_block=_post2)
        while _emitted[0] < len(NTL):
            st, wdt = NTL[_emitted[0]]
            head_slice(st, wdt)
            _emitted[0] += 1
        hstack.close()

    nc.compile()
    return nc


# ======================= host-side preparation ==============================

def _wrap16(flat):
    """idx flat [n] -> wrapped int16 [128, n//16]; pos i -> (i%16, i//16),
    replicated across the 8 Q7-core stripes."""
    n = len(flat)
    out = np.zeros((P, n // 16), np.int16)
    cols = np.arange(n) // 16
    rows = np.arange(n) % 16
    for r in range(8):
        out[r * 16 + rows, cols] = flat
    return out


def _balance_bins(deg, nbins, cap):
    """Greedy multiway partition: assign nodes to bins balancing total degree,
    each bin holding at most `cap` nodes.  Returns bin id per node."""
    import heapq
    n = len(deg)
    order = np.argsort(-deg, kind="stable")
    binid = np.empty(n, np.int32)
    counts = np.zeros(nbins, np.int32)
    heap = [(0, b) for b in range(nbins)]
    heapq.heapify(heap)
    for nd in order:
        while True:
            load, b = heapq.heappop(heap)
            if counts[b] < cap:
                break
        binid[nd] = b
        counts[b] += 1
        if counts[b] < cap:
            heapq.heappush(heap, (load + int(deg[nd]), b))
    return binid


def prep_host(x, edge_index, W1, a_src1, a_dst1, b1, W2, a_src2, a_dst2, b2,
              g, mu, world=8):
    import ml_dtypes
    x16 = np.asarray(x, np.float32).astype(np.float16)
    N = x16.shape[0]
    NBLK = int(np.ceil(N / world / P))
    CAP = NBLK * P
    nbins = world * NBLK
    PB0 = min(NBLK, HALF // (world * P))
    PB1 = NBLK - PB0

    src = np.asarray(edge_index[0]).astype(np.int64)
    dst = np.asarray(edge_index[1]).astype(np.int64)

    # --- balanced global node -> (core, block, slot) assignment
    deg = np.bincount(dst, minlength=N)
    binid = _balance_bins(deg, nbins, P)
    order = np.lexsort((np.arange(N), binid))
    # local slot position within the core's shard
    lpos = np.empty(N, np.int64)
    nxt = np.arange(nbins, dtype=np.int64) * P
    for nd in order:
        b = binid[nd]
        lpos[nd] = nxt[b]
        nxt[b] += 1
    node_core = binid // NBLK
    node_blk = binid % NBLK
    lpos -= node_core * CAP              # position within own core [0, CAP)

    # global cc table position: AllGather piece-major
    # piece0 rows: [core, blocks 0:PB0]; piece1: [core, blocks PB0:NBLK]
    in_p1 = node_blk >= PB0
    gpos = np.where(
        ~in_p1,
        node_core * (PB0 * P) + lpos,
        world * PB0 * P + node_core * (PB1 * P) + (lpos - PB0 * P))

    # per-core list of node ids in shard slot order (-1 = empty slot)
    idxmaps = []
    for c in range(world):
        m = np.full(CAP, -1, np.int64)
        mask = node_core == c
        m[lpos[mask]] = np.nonzero(mask)[0]
        idxmaps.append(m)

    # --- edges grouped by (core, block) of dst
    ecore = node_core[dst]
    eblk = node_blk[dst]
    gkey = ecore * NBLK + eblk
    gorder = np.argsort(gkey, kind="stable")
    srcg, dstg, gkeyg = src[gorder], dst[gorder], gkey[gorder]
    starts = np.concatenate(
        [[0], np.cumsum(np.bincount(gkeyg, minlength=nbins))])

    ed = {}
    CPL = CPH = 1
    for c in range(world):
        for b in range(NBLK):
            gid = c * NBLK + b
            es = srcg[starts[gid]:starts[gid + 1]]
            eds = dstg[starts[gid]:starts[gid + 1]]
            dloc = (lpos[eds] - b * P).astype(np.int64)
            tl = gpos[es]
            lo = tl < world * PB0 * P
            ed[(c, b)] = (es, tl, lo, dloc)
            CPL = max(CPL, int(np.ceil(lo.sum() / P)))
            CPH = max(CPH, int(np.ceil((~lo).sum() / P)))

    cfg = CFG(N=N, W=world, NBLK=NBLK, CPL=CPL, CPH=CPH, idxmaps=idxmaps)
    CPB = cfg.CPB
    XOFF = cfg.xoffs
    ar128 = np.arange(P, dtype=np.int64)

    def build_core(c):
        isd = np.zeros((P, NBLK * CPB * 8), np.int16)
        sth = np.zeros((P, NBLK * CPB * P), ml_dtypes.bfloat16)
        salh = np.zeros((P, NBLK * CPB * P), ml_dtypes.bfloat16)
        srcs = np.zeros(XOFF[NBLK] * P, np.int64)     # dense-chunk x rows
        for b in range(NBLK):
            es, tl, lo, dloc = ed[(c, b)]
            fl = np.zeros(CPB * P, np.int64)      # slot -> table idx (pad 0)
            fd = np.full(CPB * P, -1, np.int64)   # slot -> dst_local (pad -1)
            fs = np.zeros(CPB * P, np.int64)      # slot -> src node id
            ilo = np.where(lo)[0]
            ihi = np.where(~lo)[0]
            fl[:len(ilo)] = tl[ilo]
            fd[:len(ilo)] = dloc[ilo]
            fs[:len(ilo)] = es[ilo]
            fl[CPL * P:CPL * P + len(ihi)] = tl[ihi] - world * PB0 * P
            fd[CPL * P:CPL * P + len(ihi)] = dloc[ihi]
            fs[CPL * P:CPL * P + len(ihi)] = es[ihi]
            cb8 = b * CPB * 8
            isd[:, cb8:cb8 + CPB * 8] = _wrap16(fl)
            # one-hots from fd [CPB, P]
            fdm = fd.reshape(CPB, P)
            oh = (fdm[:, :, None] == ar128)                 # [j, e, d]
            cbp = b * CPB * P
            sth[:, cbp:cbp + CPB * P] = \
                oh.transpose(2, 0, 1).reshape(P, CPB * P)   # st[d,(j,e)]
            salh[:, cbp:cbp + CPB * P] = \
                oh.transpose(1, 0, 2).reshape(P, CPB * P)   # sall[e,(j,d)]
            # dense chunk sources: lo [CPL-dlo:CPL], hi [CPB-dhi:CPB]
            fsm = fs.reshape(CPB, P)
            dlo, dhi = cfg.dsched(b)
            o = XOFF[b] * P
            if dlo:
                srcs[o:o + dlo * P] = fsm[CPL - dlo:CPL].ravel()
            if dhi:
                srcs[o + dlo * P:o + (dlo + dhi) * P] = \
                    fsm[CPB - dhi:CPB].ravel()
        xs = x16[srcs]                            # [XOFF[-1]*P, IN]
        xsT = np.ascontiguousarray(
            xs.reshape(XOFF[NBLK], P, 2, P).transpose(3, 0, 2, 1))
        return isd, sth, salh, xsT

    # weights
    W1 = np.asarray(W1, np.float32)
    W2 = np.asarray(W2, np.float32)
    W1r = W1.reshape(H1, MD, IN)
    Ps1 = np.einsum("hdi,hd->ih", W1r, np.asarray(a_src1, np.float32))
    Pd1 = np.einsum("hdi,hd->ih", W1r, np.asarray(a_dst1, np.float32))
    W1aug = np.concatenate([W1.T, Ps1, Pd1], axis=1)
    Ps2 = W2.T @ np.asarray(a_src2, np.float32)[0][:, None]
    Pd2 = W2.T @ np.asarray(a_dst2, np.float32)[0][:, None]
    W2aug = np.concatenate([W2.T, Ps2, Pd2], axis=1)
    AUG1, AUG2 = IN + 4, IN + 2
    w1s = W1aug.reshape(2, P, AUG1).transpose(1, 0, 2).astype(np.float16)
    w2s = W2aug.reshape(2, P, AUG2).transpose(1, 0, 2).astype(np.float16)

    gm = np.asarray(g, np.float32)
    gsd = gm.reshape(2, P, KH * P).transpose(1, 0, 2).astype(np.float16)
    mu = np.asarray(mu, np.float32)
    # Gmu[f, k] = sum_m g[f, k*MD+m] * mu[k, m]
    gmu = np.einsum("fkm,km->fk", gm.reshape(IN, KH, MD), mu)
    gmud = gmu.reshape(2, P, KH).transpose(1, 0, 2).astype(np.float16)
    onesd = np.zeros((P, KH * KH), np.float16)
    for k in range(KH):
        onesd[:, k * KH + k] = 1.0
    cmu = np.linalg.norm(mu, axis=1)[:, None].astype(np.float32)
    b1b = np.broadcast_to(np.asarray(b1, np.float32), (P, HID)).copy()
    b2b = np.broadcast_to(np.asarray(b2, np.float32), (P, OUT)).copy()
    ident = np.eye(P, dtype=np.float32)
    identb = np.eye(P, dtype=ml_dtypes.bfloat16)

    shared = dict(w1s=w1s, w2s=w2s, gs=gsd, gmu=gmud, onesd=onesd, cmu=cmu,
                  b1b=b1b, b2b=b2b, ident=ident, identb=identb)
    in_maps = []
    for c in range(world):
        m = idxmaps[c]
        own = np.where(m >= 0, m, 0)
        xo = x16[own]
        xo[m < 0] = 0
        xoT = np.ascontiguousarray(
            xo.reshape(NBLK, P, 2, P).transpose(3, 0, 2, 1))
        isd_c, st_c, sal_c, xsT_c = build_core(c)
        mm = dict(shared)
        mm.update(xoTi=xoT, xsTi=xsT_c, isd=isd_c, std=st_c, sald=sal_c)
        in_maps.append(mm)
    return cfg, in_maps


def assemble(cfg, outs):
    N = cfg.N
    full = np.zeros((N, KH), np.float32)
    for c in range(cfg.W):
        o = np.asarray(outs[c]["outT"])      # [KH, SHARD_CAP]
        m = cfg.idxmaps[c]
        valid = m >= 0
        full[m[valid], :] = o[:, valid].T
    return full


_CACHE = {}


def kernel(**inputs):
    world = 8
    cfg, in_maps = prep_host(world=world, **inputs)
    key = (cfg.N, cfg.W, cfg.CPL, cfg.CPH)
    if key not in _CACHE:
        _CACHE[key] = build_program(cfg)
    nc = _CACHE[key]

    from concourse.bass_utils import run_bass_kernel_spmd
    res = run_bass_kernel_spmd(nc, in_maps, core_ids=list(range(world)))
    return assemble(cfg, res.results)


# revision 25
# speedup vs baseline: 1.0215x; 1.0215x over previous
"""Trainium2 Bass kernel for nn_NodeInference (2-layer GAT + cosine head).

v4 design (SPMD over 8 cores, dst-node sharding, hybrid gather/dense):
  Host globally re-assigns nodes to (core, block) bins, balancing per-block
  in-degree.  Both GAT layers share ONE edge-slot layout (chunks of 128
  edges per dst block, split lo/hi by global table row for int16 gather
  indices), so the per-edge one-hot matrices are built once on the host and
  shipped:
     std  [e-transposed]  st[d,(j,e)]  = (dloc[j,e]==d)   (ad lookup lhsT)
     sald                 sall[e,(j,d)] = (dloc[j,e]==d)  (scatter base)
  This removes the per-block PE broadcast + DVE is_equal chains of v2.

  P1  sharded dense: each core computes h1aug only for its OWN 6272 nodes
      -> cc1_in rows [h1|1|h2|1|as f32 x2|ad f32 x2] (768B)
  AG1 AllGather cc1_in -> cc1_out (global h1 table), in 2 pieces
  P2  edge phase 1 per dst block: chunks are HYBRID:
      - gather chunks: dma_gather rows from cc1_out (GpSimd)
      - dense chunks:  gt[:,j] = x[src_e] @ W1aug on the PE (x[src_e] is a
        host input, shipped pre-arranged per edge slot in xsTi) -- trades
        GpSimd descriptor-generation time for PE time to balance engines
      - a_d per edge = st_j^T @ adwin;  w_e = exp(min(lrelu(a_s+a_d,.2),30))
      - scatter: bp += (sall*w_h)_j^T @ rows_j  (rows carry literal 1.0 so
        the same matmul accumulates the softmax denominator)
      - epilogue -> out1T; h2aug rows -> cc2_in
      Blocks are software-pipelined (stage A: dma/dense/gather/ad of block
      b+1 emitted before stage B: ew/swh/scatter/epilogue of block b) to
      avoid in-order PE stalls.
  AG2 AllGather cc2_in -> cc2_out in 2 pieces, piece 0 issued mid-phase
  P4  edge phase 2: all chunks gathered from cc2_out (content is
      device-computed, so the host x-trick cannot apply)
  P5  head: cos sim vs mu -> outT [8, SHARD_CAP], interleaved into P4
Host scatters per-core outT into the full output via the assignment map.
"""

import sys
from dataclasses import dataclass, field
from contextlib import ExitStack

if "/opt/trn_rl_repo" not in sys.path:
    sys.path.insert(0, "/opt/trn_rl_repo")

import numpy as np

import concourse.bacc as bacc
import concourse.bass as bass
import concourse.mybir as mybir
import concourse.tile as tile
from concourse.bass import AP

P = 128
IN = 256          # input feature dim
H1 = 2            # layer-1 heads
HID = 256         # layer-1 output dim (2*128, concat)
OUT = 256         # layer-2 output dim
KH, MD = 8, 128   # cosine head shape
ROWW = 384        # fp16 cols per packed table row (768B)
HALF = 32768      # int16 table-half split
DENSE_LO = 6      # layer-1 lo chunks computed on the PE instead of gathered
DENSE_HI = 4      # layer-1 hi chunks computed on the PE
AF = mybir.ActivationFunctionType
ALU = mybir.AluOpType
DT = mybir.dt


@dataclass
class CFG:
    N: int
    W: int              # world size
    NBLK: int           # dst blocks (128 dsts) per core
    CPL: int            # lo-half chunks per block
    CPH: int
    idxmaps: object = field(default=None, repr=False)

    @property
    def SHARD_CAP(self):
        return self.NBLK * P

    @property
    def CPB(self):
        return self.CPL + self.CPH

    @property
    def DCL(self):
        return min(DENSE_LO, self.CPL)

    @property
    def DCH(self):
        return min(DENSE_HI, self.CPH)

    @property
    def DCT(self):
        return self.DCL + self.DCH

    def dsched(self, blk):
        """(dense_lo, dense_hi) for a block; the first blocks are fully
        dense so they have no AllGather dependency and run during AG1."""
        if blk < 6:
            return (self.CPL, self.CPH)
        if blk < 10:
            return (min(8, self.CPL), min(4, self.CPH))
        return (self.DCL, self.DCH)

    @property
    def xoffs(self):
        offs, t = [], 0
        for b in range(self.NBLK):
            dl, dh = self.dsched(b)
            offs.append(t)
            t += dl + dh
        offs.append(t)
        return offs

    @property
    def PB0(self):       # blocks in AllGather piece 0 (int16 row limit)
        return min(self.NBLK, HALF // (self.W * P))


def build_program(cfg: CFG):
    nc = bacc.Bacc("TRN2", target_bir_lowering=False, debug=False)
    W, NBLK = cfg.W, cfg.NBLK
    AUG1, AUG2 = IN + 4, IN + 2
    f16, bf16, f32 = DT.float16, DT.bfloat16, DT.float32
    i16 = DT.int16
    CPB, CPL, CPH = cfg.CPB, cfg.CPL, cfg.CPH
    DCL, DCH = cfg.DCL, cfg.DCH
    PB0, PB1 = cfg.PB0, cfg.NBLK - cfg.PB0
    CAP = cfg.SHARD_CAP

    with tile.TileContext(nc) as tc, ExitStack() as stack:
        dram = stack.enter_context(
            tc.tile_pool(name="dram", bufs=1, space="DRAM"))

        def din(name, shape, dtype):
            return dram.tile(shape, dtype, kind="ExternalInput", name=name,
                             uniquify=False)

        xoTi = din("xoTi", [P, NBLK, 2, P], f16)
        XOFF = cfg.xoffs
        DCTMAX = max(sum(cfg.dsched(b)) for b in range(NBLK))
        xsTi = din("xsTi", [P, XOFF[NBLK], 2, P], f16)
        isd = din("isd", [P, NBLK * CPB * 8], i16)
        std = din("std", [P, NBLK * CPB * P], bf16)
        sald = din("sald", [P, NBLK * CPB * P], bf16)
        w1s = din("w1s", [P, 2, AUG1], f16)
        w2s = din("w2s", [P, 2, AUG2], f16)
        gsd = din("gs", [P, 2, KH * P], f16)
        gmud = din("gmu", [P, 2, KH], f16)        # sum_m g[f,km]*mu[k,m]
        ond = din("onesd", [P, KH * KH], f16)     # block-diag ones
        cmu = din("cmu", [KH, 1], f32)
        b1d = din("b1b", [P, HID], f32)
        b2d = din("b2b", [P, OUT], f32)
        idn = din("ident", [P, P], f32)
        idb = din("identb", [P, P], bf16)
        outT = dram.tile([KH, CAP], f32, kind="ExternalOutput",
                         name="outT", uniquify=False)

        shsp = "Shared" if W > 1 else "Local"
        cc1_in = dram.tile([CAP, ROWW], f16, name="cc1_in")
        cc1_p0 = dram.tile([W * PB0 * P, ROWW], f16, name="cc1_p0",
                           addr_space=shsp)
        cc1_p1 = dram.tile([W * PB1 * P, ROWW], f16, name="cc1_p1",
                           addr_space=shsp)
        cc2_in = dram.tile([CAP, ROWW], f16, name="cc2_in")
        cc2_p0 = dram.tile([W * PB0 * P, ROWW], f16, name="cc2_p0",
                           addr_space=shsp)
        cc2_p1 = dram.tile([W * PB1 * P, ROWW], f16, name="cc2_p1",
                           addr_space=shsp)

        consts = stack.enter_context(tc.tile_pool(name="consts", bufs=1))
        w1_sb = consts.tile([P, 2, AUG1], f16)
        w2_sb = consts.tile([P, 2, AUG2], f16)
        g_sb = consts.tile([P, 2, KH * P], f16)
        gmu_sb = consts.tile([P, 2, KH], f16)
        on_sb = consts.tile([P, KH * KH], f16)
        cmu_sb = consts.tile([KH, 1], f32)
        b1_sb = consts.tile([P, HID], f32)
        b2_sb = consts.tile([P, OUT], f32)
        ident_sb = consts.tile([P, P], f32)
        identb_sb = consts.tile([P, P], bf16)
        out1T_sb = consts.tile([P, 2, CAP], f16)
        h2fT_sb = consts.tile([P, 2, CAP], f16)

        for dst, src in [(w1_sb, w1s), (w2_sb, w2s), (g_sb, gsd),
                         (gmu_sb, gmud), (on_sb, ond), (cmu_sb, cmu),
                         (b1_sb, b1d), (b2_sb, b2d),
                         (ident_sb, idn), (identb_sb, idb)]:
            nc.sync.dma_start(dst[:], src[:])

        def ag_piece(cin, cout_p, pc):
            """AllGather piece pc (0/1) of cin into its own Shared tensor."""
            lr = (0, PB0 * P) if pc == 0 else (PB0 * P, CAP)
            nc.gpsimd.collective_compute(
                "AllGather", ALU.bypass,
                replica_groups=[list(range(W))],
                ins=[cin[lr[0]:lr[1], :]], outs=[cout_p[:, :]])

        # ================= P1: sharded dense (own nodes only) ===============
        with tc.tile_pool(name="p1x", bufs=3) as p1x, \
             tc.tile_pool(name="p1ps", bufs=3, space="PSUM") as p1ps, \
             tc.tile_pool(name="p1row", bufs=3) as p1row:
            for blk in range(NBLK):
                xo = p1x.tile([P, 2, P], f16, tag="xo")
                nc.sync.dma_start(xo[:], xoTi[:, blk, :, :])
                ps = p1ps.tile([P, AUG1], f32, tag="ps")
                for k in range(2):
                    nc.tensor.matmul(ps[:], lhsT=xo[:, k, :],
                                     rhs=w1_sb[:, k, :],
                                     start=(k == 0), stop=(k == 1))
                # row: [h1 0:128 | 1.0 | h2 129:257 | 1.0 | as f32 x2 | ad..]
                row = p1row.tile([P, 272], f16, tag="row")
                rf32 = row[:].bitcast(f32)
                nc.scalar.activation(row[:, 0:P], ps[:, 0:P], AF.Copy)
                nc.scalar.activation(row[:, P + 1:IN + 1],
                                     ps[:, P:IN], AF.Copy)
                nc.vector.tensor_copy(rf32[:, 129:133], ps[:, IN:IN + 4])
                nc.vector.memset(row[:, P:P + 1], 1.0)
                nc.vector.memset(row[:, IN + 1:IN + 2], 1.0)
                nc.sync.dma_start(
                    cc1_in[blk * P:(blk + 1) * P, 0:IN + 10],
                    row[:, 0:IN + 10])
                if blk == PB0 - 1:
                    ag_piece(cc1_in, cc1_p0, 0)
            ag_piece(cc1_in, cc1_p1, 1)

        # ================= edge phases ======================================
        # partial L2 sums (lo sub-phase) parked per block in DRAM (f32 --
        # the exp() weights overflow f16): rows [f 0:256 | sumw]
        part_d = dram.tile([CAP, IN + 1], f32, name="part_d")

        def edge_phase(layer, part=None, post_block=None):
            if layer == 1:
                nhead = 2
                b_sb, out_t, lrelu_out = b1_sb, out1T_sb, True
                as_off, ad_off = 129, 131   # f32 col offsets in table rows
                cin = cc1_in
                c0c, c1c = 0, CPB
                with_self, do_epi = True, True
                # chunk roles are per-block (cfg.dsched): lo/hi tails are
                # dense-computed, the rest gathered
                g_rngs = d_rngs = True   # per-block, built in stage_a
            else:
                nhead = 1
                b_sb, out_t, lrelu_out = b2_sb, h2fT_sb, False
                as_off, ad_off = 129, 130
                cin = cc2_in
                d_rngs = []
                SPC = min(CPL, 8)   # pass-A chunk count (one gather call)
                if part == 'lo':
                    c0c, c1c = 0, SPC
                    with_self, do_epi = False, False
                    g_rngs = [(0, SPC, cc2_p0)]
                else:
                    c0c, c1c = SPC, CPB
                    with_self, do_epi = True, True
                    g_rngs = [(SPC, CPL, cc2_p0), (CPL, CPB, cc2_p1)]
            NCF = c1c - c0c
            sfx = f"{layer}{part or ''}"

            estack = ExitStack()
            pi = estack.enter_context(
                tc.tile_pool(name=f"idx{sfx}", bufs=4))
            pso = estack.enter_context(
                tc.tile_pool(name=f"soh{sfx}", bufs=4))
            pg = estack.enter_context(
                tc.tile_pool(name=f"gath{sfx}", bufs=4))
            pw = estack.enter_context(
                tc.tile_pool(name=f"win{sfx}", bufs=4))
            px = (estack.enter_context(tc.tile_pool(name="dsx", bufs=2))
                  if d_rngs else None)
            pps = (estack.enter_context(
                tc.tile_pool(name="dsps", bufs=2, space="PSUM"))
                if d_rngs else None)
            pad_ = estack.enter_context(
                tc.tile_pool(name=f"adps{sfx}", bufs=2, space="PSUM"))
            pe_ = estack.enter_context(tc.tile_pool(name=f"ew{sfx}", bufs=4))
            pm = estack.enter_context(tc.tile_pool(name=f"sw{sfx}", bufs=2))
            pp = estack.enter_context(
                tc.tile_pool(name=f"bps{sfx}", bufs=2, space="PSUM"))
            pt = (estack.enter_context(
                tc.tile_pool(name=f"tps{sfx}", bufs=1, space="PSUM"))
                if do_epi else None)
            po = estack.enter_context(tc.tile_pool(name=f"epi{sfx}", bufs=2))
            ph = (estack.enter_context(
                tc.tile_pool(name=f"h2ps{sfx}", bufs=1, space="PSUM"))
                if layer == 1 else None)

            NSW = NCF + (1 if with_self else 0)
            state = {}

            def stage_a(blk):
                cb8 = blk * CPB * 8
                cbp = blk * CPB * P
                isdt = pi.tile([P, NCF * 8], i16, tag="isd")
                nc.sync.dma_start(isdt[:], isd[:, cb8 + c0c * 8:
                                                cb8 + c1c * 8])
                st = pso.tile([P, NCF * P], bf16, tag="st")
                nc.sync.dma_start(st[:], std[:, cbp + c0c * P:
                                             cbp + c1c * P])
                sall = pso.tile([P, NCF * P], bf16, tag="sall")
                nc.sync.dma_start(sall[:], sald[:, cbp + c0c * P:
                                                cbp + c1c * P])
                win = pw.tile([P, ROWW], f16, tag="win")
                nc.sync.dma_start(win[:], cin[blk * P:(blk + 1) * P, :])
                winf = win[:].bitcast(f32)

                gt = pg.tile([P, NCF, ROWW], f16, tag="gt")
                gtf = gt[:].bitcast(f32)
                if layer == 1:
                    dlo, dhi = cfg.dsched(blk)
                    g_rb = [(0, CPL - dlo, cc1_p0), (CPL, CPB - dhi, cc1_p1)]
                    d_rb = [(CPL - dlo, CPL, 0), (CPB - dhi, CPB, dlo)]
                else:
                    g_rb, d_rb = g_rngs, []
                # gather chunks (chunk j maps to local slot j - c0c)
                MXC = 8
                for r0, r1, tab in g_rb:
                    if r1 <= r0:
                        continue
                    for c0 in range(r0, r1, MXC):
                        c1 = min(c0 + MXC, r1)
                        nc.gpsimd.dma_gather(
                            gt[:, c0 - c0c:c1 - c0c, :], tab[:, :],
                            isdt[:, (c0 - c0c) * 8:(c1 - c0c) * 8],
                            (c1 - c0) * P, (c1 - c0) * P, ROWW)
                # dense chunks (layer 1 only)
                if d_rb:
                    dct_b = sum(cfg.dsched(blk))
                    xt = px.tile([P, DCTMAX, 2, P], f16, tag="xt")
                    xo = XOFF[blk]
                    nc.sync.dma_start(xt[:, 0:dct_b, :, :],
                                      xsTi[:, xo:xo + dct_b, :, :])
                    for r0, r1, xb in d_rb:
                        for j in range(r0, r1):
                            ps = pps.tile([P, AUG1], f32, tag="ps")
                            xi = xb + (j - r0)
                            for k in range(2):
                                nc.tensor.matmul(ps[:], lhsT=xt[:, xi, k, :],
                                                 rhs=w1_sb[:, k, :],
                                                 start=(k == 0), stop=(k == 1))
                            nc.scalar.activation(gt[:, j, 0:P], ps[:, 0:P],
                                                 AF.Copy)
                            nc.scalar.activation(gt[:, j, P + 1:IN + 1],
                                                 ps[:, P:IN], AF.Copy)
                            nc.vector.tensor_copy(gtf[:, j, 129:131],
                                                  ps[:, IN:IN + 2])
                        nc.vector.memset(gt[:, r0:r1, P:P + 1], 1.0)
                        nc.vector.memset(gt[:, r0:r1, IN + 1:IN + 2], 1.0)

                # a_d per edge: ad_ps[e, (j,h)] = st_j^T @ adwin
                adw16 = pe_.tile([P, nhead], f16, tag="adw16")
                nc.vector.tensor_copy(adw16[:],
                                      winf[:, ad_off:ad_off + nhead])
                ad_ps = pad_.tile([P, NCF * nhead], f32, tag="adps")
                for j in range(NCF):
                    nc.tensor.matmul(ad_ps[:, j * nhead:(j + 1) * nhead],
                                     lhsT=st[:, j * P:(j + 1) * P],
                                     rhs=adw16[:], start=(j == 0),
                                     stop=(j == NCF - 1))
                pl = None
                if layer == 2 and do_epi:
                    pl = po.tile([P, IN + 1], f32, tag="pl")
                    nc.sync.dma_start(pl[:],
                                      part_d[blk * P:(blk + 1) * P, :])
                state[blk] = (gt, gtf, win, winf, sall, ad_ps, pl)

            def stage_b(blk):
                gt, gtf, win, winf, sall, ad_ps, pl = state.pop(blk)
                # edge weights w = exp(lrelu(a_s + a_d, .2))
                ew = pe_.tile([P, NCF * nhead], f32, tag="ew")
                wv = pe_.tile([P, NCF * nhead], bf16, tag="wv")
                as_v = gtf[:, :, as_off:as_off + nhead]
                nc.vector.tensor_tensor(ew[:], as_v, ad_ps[:], op=ALU.add)
                ewl = pe_.tile([P, NCF * nhead], f32, tag="ewl")
                nc.vector.scalar_tensor_tensor(
                    out=ewl[:], in0=ew[:], scalar=0.2, in1=ew[:],
                    op0=ALU.mult, op1=ALU.max)
                nc.scalar.activation(wv[:], ewl[:], AF.Exp)

                if with_self:
                    # self-loop weights from the window rows
                    ws = pe_.tile([P, nhead], f32, tag="ws")
                    nc.vector.tensor_tensor(
                        ws[:], winf[:, as_off:as_off + nhead],
                        winf[:, ad_off:ad_off + nhead], op=ALU.add)
                    wt = pe_.tile([P, nhead], f32, tag="wt")
                    nc.vector.scalar_tensor_tensor(
                        out=wt[:], in0=ws[:], scalar=0.2, in1=ws[:],
                        op0=ALU.mult, op1=ALU.max)
                    nc.scalar.activation(wt[:], wt[:], AF.Exp)

                # weighted one-hot scatter: sw_h = sall * w_h (stride-0 bcast)
                swh = []
                for h in range(nhead):
                    wvh = wv[:, h:h + 1]
                    wview = AP(wvh.tensor, wvh.offset,
                               [wvh.ap[0], [nhead, NCF], [0, P]])
                    swt = pm.tile([P, NCF * P], bf16, tag=f"swh{h}")
                    nc.vector.tensor_tensor(swt[:], sall[:], wview,
                                            op=ALU.mult)
                    swh.append(swt)

                # single psum accumulation group across all j/h matmuls:
                # start only on the first, stop only on the very last.  Table
                # rows carry a literal 1.0 after each head's features, so one
                # matmul accumulates both the weighted feature sum and the
                # softmax denominator:
                # bp layout [f1 0:128 | sumw1 | f2 129:257 | sumw2] (2 heads)
                # or [f 0:256 | sumw] (1 head).
                HB = P + 1 if nhead == 2 else IN + 1
                bp = pp.tile([P, nhead * HB], f32, tag="bp")
                for j in range(NSW):
                    last = (j == NSW - 1)
                    selfc = with_self and (j == NCF)
                    for h in range(nhead):
                        if selfc:
                            sw = pm.tile([P, P], bf16, tag="sw")
                            nc.vector.tensor_scalar(
                                out=sw[:], in0=identb_sb[:],
                                scalar1=wt[:, h:h + 1],
                                scalar2=None, op0=ALU.mult)
                            lhsT = sw[:]
                        else:
                            lhsT = swh[h][:, j * P:(j + 1) * P]
                        c0, c1 = h * HB, (h + 1) * HB
                        rhs = win[:, c0:c1] if selfc else gt[:, j, c0:c1]
                        nc.tensor.matmul(bp[:, c0:c1], lhsT=lhsT,
                                         rhs=rhs,
                                         start=(j == 0 and h == 0),
                                         stop=(last and h == nhead - 1))

                if not do_epi:
                    # L2 lo sub-phase: park the partial sums, finish later
                    pf = po.tile([P, IN + 1], f32, tag="pf")
                    nc.scalar.activation(pf[:], bp[:], AF.Copy)
                    nc.sync.dma_start(part_d[blk * P:(blk + 1) * P, :],
                                      pf[:])
                    if post_block is not None:
                        post_block(blk)
                    return

                # ---- block epilogue
                if layer == 2:
                    tb = po.tile([P, IN + 1], f32, tag="tb")
                    nc.vector.tensor_tensor(tb[:], bp[:], pl[:],
                                            op=ALU.add)
                    bpv = tb
                else:
                    bpv = bp
                rec = po.tile([P, nhead], f32, tag="rec")
                for h in range(nhead):
                    nc.vector.reciprocal(rec[:, h:h + 1],
                                         bpv[:, (h + 1) * HB - 1:
                                             (h + 1) * HB])
                ti = po.tile([P, IN], f32, tag="ti")
                if nhead == 2:
                    nc.scalar.activation(ti[:, 0:P], bpv[:, 0:P], AF.Copy,
                                         scale=rec[:, 0:1])
                    nc.scalar.activation(ti[:, P:IN], bpv[:, HB:HB + P],
                                         AF.Copy, scale=rec[:, 1:2])
                else:
                    nc.scalar.activation(ti[:], bpv[:, 0:IN], AF.Copy,
                                         scale=rec[:, 0:1])
                nc.vector.tensor_tensor(ti[:], ti[:], b_sb[:], op=ALU.add)
                if lrelu_out:
                    # lrelu(x) = 0.01x + relu(0.99x)
                    tr = po.tile([P, IN], f32, tag="tr")
                    nc.scalar.activation(tr[:], ti[:], AF.Relu, scale=0.99)
                    nc.vector.scalar_tensor_tensor(
                        out=ti[:], in0=ti[:], scalar=0.01, in1=tr[:],
                        op0=ALU.mult, op1=ALU.add)
                for k in range(2):
                    tp = pt.tile([P, P], f32, tag="tp")
                    nc.tensor.transpose(tp[:], ti[:, k * P:(k + 1) * P],
                                        ident_sb[:])
                    nc.scalar.activation(out_t[:, k, blk * P:(blk + 1) * P],
                                         tp[:], AF.Copy)

                if layer == 1:
                    hp = ph.tile([P, AUG2], f32, tag="hp")
                    for k in range(2):
                        nc.tensor.matmul(
                            hp[:],
                            lhsT=out1T_sb[:, k, blk * P:(blk + 1) * P],
                            rhs=w2_sb[:, k, :], start=(k == 0), stop=(k == 1))
                    # L2 row: [h 0:256 | 1.0 | a_s f32 @129 | a_d f32 @130]
                    row2 = po.tile([P, ROWW], f16, tag="row2")
                    nc.scalar.activation(row2[:, 0:OUT], hp[:, 0:OUT], AF.Copy)
                    nc.vector.memset(row2[:, OUT:OUT + 2], 1.0)
                    r2f = row2[:].bitcast(f32)
                    nc.vector.tensor_copy(r2f[:, 129:131],
                                          hp[:, OUT:OUT + 2])
                    nc.sync.dma_start(
                        cc2_in[blk * P:(blk + 1) * P, 0:OUT + 6],
                        row2[:, 0:OUT + 6])

                if post_block is not None:
                    post_block(blk)

            for b in range(NBLK + 2):
                if b >= 2:
                    stage_b(b - 2)
                if b < NBLK:
                    stage_a(b)

            estack.close()

        def _post1(blk):
            if blk == PB0 - 1:
                ag_piece(cc2_in, cc2_p0, 0)
            elif blk == NBLK - 1:
                ag_piece(cc2_in, cc2_p1, 1)

        edge_phase(1, post_block=_post1)
        edge_phase(2, part='lo')

        # ====== P5 head, interleaved into edge phase 2 ======================
        hstack = ExitStack()
        hps = hstack.enter_context(
            tc.tile_pool(name="hps", bufs=1, space="PSUM"))
        hsb = hstack.enter_context(tc.tile_pool(name="hsb", bufs=2))
        sps = hstack.enter_context(
            tc.tile_pool(name="sps", bufs=1, space="PSUM"))
        hepi = hstack.enter_context(tc.tile_pool(name="hepi", bufs=2))

        NTL = []
        _st = 0
        while _st < CAP:
            _w = min(512 if _st + 1024 <= CAP else 256, CAP - _st)
            NTL.append((_st, _w))
            _st += _w

        def head_slice(st, wdt):
            nump = sps.tile([KH, 512], f32, tag="nump")
            nrmp = sps.tile([KH, 512], f32, tag="nrmp")
            # numerator: num[k,n] = sum_f Gmu[f,k] * h2[f,n]
            for f in range(2):
                nc.tensor.matmul(nump[:, 0:wdt],
                                 lhsT=gmu_sb[:, f, :],
                                 rhs=h2fT_sb[:, f, st:st + wdt],
                                 start=(f == 0), stop=(f == 1))
            for k in range(KH):
                hp = hps.tile([P, 512], f32, tag="hp")
                for f in range(2):
                    nc.tensor.matmul(hp[:, 0:wdt],
                                     lhsT=g_sb[:, f, k * P:(k + 1) * P],
                                     rhs=h2fT_sb[:, f, st:st + wdt],
                                     start=(f == 0), stop=(f == 1))
                sq16 = hsb.tile([P, 512], f16, tag="sq16")
                nc.scalar.activation(sq16[:, 0:wdt], hp[:, 0:wdt], AF.Square)
                nc.tensor.matmul(nrmp[:, 0:wdt],
                                 lhsT=on_sb[:, k * KH:(k + 1) * KH],
                                 rhs=sq16[:, 0:wdt], start=(k == 0),
                                 stop=(k == KH - 1))
            sq = hepi.tile([KH, 512], f32, tag="sqr")
            # sqrt(x) = exp(0.5*ln(x)) -- keeps ACT on the ln/exp table set
            nc.scalar.activation(sq[:, 0:wdt], nrmp[:, 0:wdt], AF.Ln)
            nc.scalar.activation(sq[:, 0:wdt], sq[:, 0:wdt], AF.Exp,
                                 scale=0.5)
            nc.vector.tensor_scalar(out=sq[:, 0:wdt], in0=sq[:, 0:wdt],
                                    scalar1=cmu_sb[:, 0:1], scalar2=1e-8,
                                    op0=ALU.mult, op1=ALU.max)
            nc.vector.reciprocal(sq[:, 0:wdt], sq[:, 0:wdt])
            res = hepi.tile([KH, 512], f32, tag="res")
            nc.vector.tensor_tensor(res[:, 0:wdt], nump[:, 0:wdt],
                                    sq[:, 0:wdt], op=ALU.mult)
            nc.sync.dma_start(outT[:, st:st + wdt], res[:, 0:wdt])

        _emitted = [0]

        def _post2(blk):
            done = (blk + 1) * P
            while _emitted[0] < len(NTL):
                st, wdt = NTL[_emitted[0]]
                if st + wdt > done:
                    break
                head_slice(st, wdt)
                _emitted[0] += 1

        edge_phase(2, part='hi', post_block=_post2)
        while _emitted[0] < len(NTL):
            st, wdt = NTL[_emitted[0]]
            head_slice(st, wdt)
            _emitted[0] += 1
        hstack.close()

    nc.compile()
    return nc


# ======================= host-side preparation ==============================

def _wrap16(flat):
    """idx flat [n] -> wrapped int16 [128, n//16]; pos i -> (i%16, i//16),
    replicated across the 8 Q7-core stripes."""
    n = len(flat)
    out = np.zeros((P, n // 16), np.int16)
    cols = np.arange(n) // 16
    rows = np.arange(n) % 16
    for r in range(8):
        out[r * 16 + rows, cols] = flat
    return out


def _balance_bins(deg, nbins, cap):
    """Greedy multiway partition: assign nodes to bins balancing total degree,
    each bin holding at most `cap` nodes.  Returns bin id per node."""
    import heapq
    n = len(deg)
    order = np.argsort(-deg, kind="stable")
    binid = np.empty(n, np.int32)
    counts = np.zeros(nbins, np.int32)
    heap = [(0, b) for b in range(nbins)]
    heapq.heapify(heap)
    for nd in order:
        while True:
            load, b = heapq.heappop(heap)
            if counts[b] < cap:
                break
        binid[nd] = b
        counts[b] += 1
        if counts[b] < cap:
            heapq.heappush(heap, (load + int(deg[nd]), b))
    return binid


def prep_host(x, edge_index, W1, a_src1, a_dst1, b1, W2, a_src2, a_dst2, b2,
              g, mu, world=8):
    import ml_dtypes
    x16 = np.asarray(x, np.float32).astype(np.float16)
    N = x16.shape[0]
    NBLK = int(np.ceil(N / world / P))
    CAP = NBLK * P
    nbins = world * NBLK
    PB0 = min(NBLK, HALF // (world * P))
    PB1 = NBLK - PB0

    src = np.asarray(edge_index[0]).astype(np.int64)
    dst = np.asarray(edge_index[1]).astype(np.int64)

    # --- balanced global node -> (core, block, slot) assignment
    deg = np.bincount(dst, minlength=N)
    binid = _balance_bins(deg, nbins, P)
    order = np.lexsort((np.arange(N), binid))
    # local slot position within the core's shard
    lpos = np.empty(N, np.int64)
    nxt = np.arange(nbins, dtype=np.int64) * P
    for nd in order:
        b = binid[nd]
        lpos[nd] = nxt[b]
        nxt[b] += 1
    node_core = binid // NBLK
    node_blk = binid % NBLK
    lpos -= node_core * CAP              # position within own core [0, CAP)

    # global cc table position: AllGather piece-major
    # piece0 rows: [core, blocks 0:PB0]; piece1: [core, blocks PB0:NBLK]
    in_p1 = node_blk >= PB0
    gpos = np.where(
        ~in_p1,
        node_core * (PB0 * P) + lpos,
        world * PB0 * P + node_core * (PB1 * P) + (lpos - PB0 * P))

    # per-core list of node ids in shard slot order (-1 = empty slot)
    idxmaps = []
    for c in range(world):
        m = np.full(CAP, -1, np.int64)
        mask = node_core == c
        m[lpos[mask]] = np.nonzero(mask)[0]
        idxmaps.append(m)

    # --- edges grouped by (core, block) of dst
    ecore = node_core[dst]
    eblk = node_blk[dst]
    gkey = ecore * NBLK + eblk
    gorder = np.argsort(gkey, kind="stable")
    srcg, dstg, gkeyg = src[gorder], dst[gorder], gkey[gorder]
    starts = np.concatenate(
        [[0], np.cumsum(np.bincount(gkeyg, minlength=nbins))])

    ed = {}
    CPL = CPH = 1
    for c in range(world):
        for b in range(NBLK):
            gid = c * NBLK + b
            es = srcg[starts[gid]:starts[gid + 1]]
            eds = dstg[starts[gid]:starts[gid + 1]]
            dloc = (lpos[eds] - b * P).astype(np.int64)
            tl = gpos[es]
            lo = tl < world * PB0 * P
            ed[(c, b)] = (es, tl, lo, dloc)
            CPL = max(CPL, int(np.ceil(lo.sum() / P)))
            CPH = max(CPH, int(np.ceil((~lo).sum() / P)))

    cfg = CFG(N=N, W=world, NBLK=NBLK, CPL=CPL, CPH=CPH, idxmaps=idxmaps)
    CPB = cfg.CPB
    XOFF = cfg.xoffs
    ar128 = np.arange(P, dtype=np.int64)

    def build_core(c):
        isd = np.zeros((P, NBLK * CPB * 8), np.int16)
        sth = np.zeros((P, NBLK * CPB * P), ml_dtypes.bfloat16)
        salh = np.zeros((P, NBLK * CPB * P), ml_dtypes.bfloat16)
        srcs = np.zeros(XOFF[NBLK] * P, np.int64)     # dense-chunk x rows
        for b in range(NBLK):
            es, tl, lo, dloc = ed[(c, b)]
            fl = np.zeros(CPB * P, np.int64)      # slot -> table idx (pad 0)
            fd = np.full(CPB * P, -1, np.int64)   # slot -> dst_local (pad -1)
            fs = np.zeros(CPB * P, np.int64)      # slot -> src node id
            ilo = np.where(lo)[0]
            ihi = np.where(~lo)[0]
            fl[:len(ilo)] = tl[ilo]
            fd[:len(ilo)] = dloc[ilo]
            fs[:len(ilo)] = es[ilo]
            fl[CPL * P:CPL * P + len(ihi)] = tl[ihi] - world * PB0 * P
            fd[CPL * P:CPL * P + len(ihi)] = dloc[ihi]
            fs[CPL * P:CPL * P + len(ihi)] = es[ihi]
            cb8 = b * CPB * 8
            isd[:, cb8:cb8 + CPB * 8] = _wrap16(fl)
            # one-hots from fd [CPB, P]
            fdm = fd.reshape(CPB, P)
            oh = (fdm[:, :, None] == ar128)                 # [j, e, d]
            cbp = b * CPB * P
            sth[:, cbp:cbp + CPB * P] = \
                oh.transpose(2, 0, 1).reshape(P, CPB * P)   # st[d,(j,e)]
            salh[:, cbp:cbp + CPB * P] = \
                oh.transpose(1, 0, 2).reshape(P, CPB * P)   # sall[e,(j,d)]
            # dense chunk sources: lo [CPL-dlo:CPL], hi [CPB-dhi:CPB]
            fsm = fs.reshape(CPB, P)
            dlo, dhi = cfg.dsched(b)
            o = XOFF[b] * P
            if dlo:
                srcs[o:o + dlo * P] = fsm[CPL - dlo:CPL].ravel()
            if dhi:
                srcs[o + dlo * P:o + (dlo + dhi) * P] = \
                    fsm[CPB - dhi:CPB].ravel()
        xs = x16[srcs]                            # [XOFF[-1]*P, IN]
        xsT = np.ascontiguousarray(
            xs.reshape(XOFF[NBLK], P, 2, P).transpose(3, 0, 2, 1))
        return isd, sth, salh, xsT

    # weights
    W1 = np.asarray(W1, np.float32)
    W2 = np.asarray(W2, np.float32)
    W1r = W1.reshape(H1, MD, IN)
    Ps1 = np.einsum("hdi,hd->ih", W1r, np.asarray(a_src1, np.float32))
    Pd1 = np.einsum("hdi,hd->ih", W1r, np.asarray(a_dst1, np.float32))
    W1aug = np.concatenate([W1.T, Ps1, Pd1], axis=1)
    Ps2 = W2.T @ np.asarray(a_src2, np.float32)[0][:, None]
    Pd2 = W2.T @ np.asarray(a_dst2, np.float32)[0][:, None]
    W2aug = np.concatenate([W2.T, Ps2, Pd2], axis=1)
    AUG1, AUG2 = IN + 4, IN + 2
    w1s = W1aug.reshape(2, P, AUG1).transpose(1, 0, 2).astype(np.float16)
    w2s = W2aug.reshape(2, P, AUG2).transpose(1, 0, 2).astype(np.float16)

    gm = np.asarray(g, np.float32)
    gsd = gm.reshape(2, P, KH * P).transpose(1, 0, 2).astype(np.float16)
    mu = np.asarray(mu, np.float32)
    # Gmu[f, k] = sum_m g[f, k*MD+m] * mu[k, m]
    gmu = np.einsum("fkm,km->fk", gm.reshape(IN, KH, MD), mu)
    gmud = gmu.reshape(2, P, KH).transpose(1, 0, 2).astype(np.float16)
    onesd = np.zeros((P, KH * KH), np.float16)
    for k in range(KH):
        onesd[:, k * KH + k] = 1.0
    cmu = np.linalg.norm(mu, axis=1)[:, None].astype(np.float32)
    b1b = np.broadcast_to(np.asarray(b1, np.float32), (P, HID)).copy()
    b2b = np.broadcast_to(np.asarray(b2, np.float32), (P, OUT)).copy()
    ident = np.eye(P, dtype=np.float32)
    identb = np.eye(P, dtype=ml_dtypes.bfloat16)

    shared = dict(w1s=w1s, w2s=w2s, gs=gsd, gmu=gmud, onesd=onesd, cmu=cmu,
                  b1b=b1b, b2b=b2b, ident=ident, identb=identb)
    in_maps = []
    for c in range(world):
        m = idxmaps[c]
        own = np.where(m >= 0, m, 0)
        xo = x16[own]
        xo[m < 0] = 0
        xoT = np.ascontiguousarray(
            xo.reshape(NBLK, P, 2, P).transpose(3, 0, 2, 1))
        isd_c, st_c, sal_c, xsT_c = build_core(c)
        mm = dict(shared)
        mm.update(xoTi=xoT, xsTi=xsT_c, isd=isd_c, std=st_c, sald=sal_c)
        in_maps.append(mm)
    return cfg, in_maps


def assemble(cfg, outs):
    N = cfg.N
    full = np.zeros((N, KH), np.float32)
    for c in range(cfg.W):
        o = np.asarray(outs[c]["outT"])      # [KH, SHARD_CAP]
        m = cfg.idxmaps[c]
        valid = m >= 0
        full[m[valid], :] = o[:, valid].T
    return full


_CACHE = {}


def kernel(**inputs):
    world = 8
    cfg, in_maps = prep_host(world=world, **inputs)
    key = (cfg.N, cfg.W, cfg.CPL, cfg.CPH)
    if key not in _CACHE:
        _CACHE[key] = build_program(cfg)
    nc = _CACHE[key]

    from concourse.bass_utils import run_bass_kernel_spmd
    res = run_bass_kernel_spmd(nc, in_maps, core_ids=list(range(world)))
    return assemble(cfg, res.results)


# revision 26
# speedup vs baseline: 1.0290x; 1.0073x over previous
"""Trainium2 Bass kernel for nn_NodeInference (2-layer GAT + cosine head).

v4 design (SPMD over 8 cores, dst-node sharding, hybrid gather/dense):
  Host globally re-assigns nodes to (core, block) bins, balancing per-block
  in-degree.  Both GAT layers share ONE edge-slot layout (chunks of 128
  edges per dst block, split lo/hi by global table row for int16 gather
  indices), so the per-edge one-hot matrices are built once on the host and
  shipped:
     std  [e-transposed]  st[d,(j,e)]  = (dloc[j,e]==d)   (ad lookup lhsT)
     sald                 sall[e,(j,d)] = (dloc[j,e]==d)  (scatter base)
  This removes the per-block PE broadcast + DVE is_equal chains of v2.

  P1  sharded dense: each core computes h1aug only for its OWN 6272 nodes
      -> cc1_in rows [h1|1|h2|1|as f32 x2|ad f32 x2] (768B)
  AG1 AllGather cc1_in -> cc1_out (global h1 table), in 2 pieces
  P2  edge phase 1 per dst block: chunks are HYBRID:
      - gather chunks: dma_gather rows from cc1_out (GpSimd)
      - dense chunks:  gt[:,j] = x[src_e] @ W1aug on the PE (x[src_e] is a
        host input, shipped pre-arranged per edge slot in xsTi) -- trades
        GpSimd descriptor-generation time for PE time to balance engines
      - a_d per edge = st_j^T @ adwin;  w_e = exp(min(lrelu(a_s+a_d,.2),30))
      - scatter: bp += (sall*w_h)_j^T @ rows_j  (rows carry literal 1.0 so
        the same matmul accumulates the softmax denominator)
      - epilogue -> out1T; h2aug rows -> cc2_in
      Blocks are software-pipelined (stage A: dma/dense/gather/ad of block
      b+1 emitted before stage B: ew/swh/scatter/epilogue of block b) to
      avoid in-order PE stalls.
  AG2 AllGather cc2_in -> cc2_out in 2 pieces, piece 0 issued mid-phase
  P4  edge phase 2: all chunks gathered from cc2_out (content is
      device-computed, so the host x-trick cannot apply)
  P5  head: cos sim vs mu -> outT [8, SHARD_CAP], interleaved into P4
Host scatters per-core outT into the full output via the assignment map.
"""

import sys
from dataclasses import dataclass, field
from contextlib import ExitStack

if "/opt/trn_rl_repo" not in sys.path:
    sys.path.insert(0, "/opt/trn_rl_repo")

import numpy as np

import concourse.bacc as bacc
import concourse.bass as bass
import concourse.mybir as mybir
import concourse.tile as tile
from concourse.bass import AP

P = 128
IN = 256          # input feature dim
H1 = 2            # layer-1 heads
HID = 256         # layer-1 output dim (2*128, concat)
OUT = 256         # layer-2 output dim
KH, MD = 8, 128   # cosine head shape
ROWW = 384        # fp16 cols per packed table row (768B)
HALF = 32768      # int16 table-half split
DENSE_LO = 5      # layer-1 lo chunks computed on the PE instead of gathered
DENSE_HI = 4      # layer-1 hi chunks computed on the PE
AF = mybir.ActivationFunctionType
ALU = mybir.AluOpType
DT = mybir.dt


@dataclass
class CFG:
    N: int
    W: int              # world size
    NBLK: int           # dst blocks (128 dsts) per core
    CPL: int            # lo-half chunks per block
    CPH: int
    idxmaps: object = field(default=None, repr=False)

    @property
    def SHARD_CAP(self):
        return self.NBLK * P

    @property
    def CPB(self):
        return self.CPL + self.CPH

    @property
    def DCL(self):
        return min(DENSE_LO, self.CPL)

    @property
    def DCH(self):
        return min(DENSE_HI, self.CPH)

    @property
    def DCT(self):
        return self.DCL + self.DCH

    def dsched(self, blk):
        """(dense_lo, dense_hi) for a block; the first blocks are fully
        dense so they have no AllGather dependency and run during AG1."""
        if blk < 6:
            return (self.CPL, self.CPH)
        if blk < 10:
            return (min(8, self.CPL), min(4, self.CPH))
        return (self.DCL, self.DCH)

    @property
    def xoffs(self):
        offs, t = [], 0
        for b in range(self.NBLK):
            dl, dh = self.dsched(b)
            offs.append(t)
            t += dl + dh
        offs.append(t)
        return offs

    @property
    def PB0(self):       # blocks in AllGather piece 0 (int16 row limit)
        return min(self.NBLK, HALF // (self.W * P))


def build_program(cfg: CFG):
    nc = bacc.Bacc("TRN2", target_bir_lowering=False, debug=False)
    W, NBLK = cfg.W, cfg.NBLK
    AUG1, AUG2 = IN + 4, IN + 2
    f16, bf16, f32 = DT.float16, DT.bfloat16, DT.float32
    i16 = DT.int16
    CPB, CPL, CPH = cfg.CPB, cfg.CPL, cfg.CPH
    DCL, DCH = cfg.DCL, cfg.DCH
    PB0, PB1 = cfg.PB0, cfg.NBLK - cfg.PB0
    CAP = cfg.SHARD_CAP

    with tile.TileContext(nc) as tc, ExitStack() as stack:
        dram = stack.enter_context(
            tc.tile_pool(name="dram", bufs=1, space="DRAM"))

        def din(name, shape, dtype):
            return dram.tile(shape, dtype, kind="ExternalInput", name=name,
                             uniquify=False)

        xoTi = din("xoTi", [P, NBLK, 2, P], f16)
        XOFF = cfg.xoffs
        DCTMAX = max(sum(cfg.dsched(b)) for b in range(NBLK))
        xsTi = din("xsTi", [P, XOFF[NBLK], 2, P], f16)
        isd = din("isd", [P, NBLK * CPB * 8], i16)
        std = din("std", [P, NBLK * CPB * P], bf16)
        sald = din("sald", [P, NBLK * CPB * P], bf16)
        w1s = din("w1s", [P, 2, AUG1], f16)
        w2s = din("w2s", [P, 2, AUG2], f16)
        gsd = din("gs", [P, 2, KH * P], f16)
        gmud = din("gmu", [P, 2, KH], f16)        # sum_m g[f,km]*mu[k,m]
        ond = din("onesd", [P, KH * KH], f16)     # block-diag ones
        cmu = din("cmu", [KH, 1], f32)
        b1d = din("b1b", [P, HID], f32)
        b2d = din("b2b", [P, OUT], f32)
        idn = din("ident", [P, P], f32)
        idb = din("identb", [P, P], bf16)
        outT = dram.tile([KH, CAP], f32, kind="ExternalOutput",
                         name="outT", uniquify=False)

        shsp = "Shared" if W > 1 else "Local"
        cc1_in = dram.tile([CAP, ROWW], f16, name="cc1_in")
        cc1_p0 = dram.tile([W * PB0 * P, ROWW], f16, name="cc1_p0",
                           addr_space=shsp)
        cc1_p1 = dram.tile([W * PB1 * P, ROWW], f16, name="cc1_p1",
                           addr_space=shsp)
        cc2_in = dram.tile([CAP, ROWW], f16, name="cc2_in")
        cc2_p0 = dram.tile([W * PB0 * P, ROWW], f16, name="cc2_p0",
                           addr_space=shsp)
        cc2_p1 = dram.tile([W * PB1 * P, ROWW], f16, name="cc2_p1",
                           addr_space=shsp)

        consts = stack.enter_context(tc.tile_pool(name="consts", bufs=1))
        w1_sb = consts.tile([P, 2, AUG1], f16)
        w2_sb = consts.tile([P, 2, AUG2], f16)
        g_sb = consts.tile([P, 2, KH * P], f16)
        gmu_sb = consts.tile([P, 2, KH], f16)
        on_sb = consts.tile([P, KH * KH], f16)
        cmu_sb = consts.tile([KH, 1], f32)
        b1_sb = consts.tile([P, HID], f32)
        b2_sb = consts.tile([P, OUT], f32)
        ident_sb = consts.tile([P, P], f32)
        identb_sb = consts.tile([P, P], bf16)
        out1T_sb = consts.tile([P, 2, CAP], f16)
        h2fT_sb = consts.tile([P, 2, CAP], f16)

        for dst, src in [(w1_sb, w1s), (w2_sb, w2s), (g_sb, gsd),
                         (gmu_sb, gmud), (on_sb, ond), (cmu_sb, cmu),
                         (b1_sb, b1d), (b2_sb, b2d),
                         (ident_sb, idn), (identb_sb, idb)]:
            nc.sync.dma_start(dst[:], src[:])

        def ag_piece(cin, cout_p, pc):
            """AllGather piece pc (0/1) of cin into its own Shared tensor."""
            lr = (0, PB0 * P) if pc == 0 else (PB0 * P, CAP)
            nc.gpsimd.collective_compute(
                "AllGather", ALU.bypass,
                replica_groups=[list(range(W))],
                ins=[cin[lr[0]:lr[1], :]], outs=[cout_p[:, :]])

        # ================= P1: sharded dense (own nodes only) ===============
        with tc.tile_pool(name="p1x", bufs=3) as p1x, \
             tc.tile_pool(name="p1ps", bufs=3, space="PSUM") as p1ps, \
             tc.tile_pool(name="p1row", bufs=3) as p1row:
            for blk in range(NBLK):
                xo = p1x.tile([P, 2, P], f16, tag="xo")
                nc.sync.dma_start(xo[:], xoTi[:, blk, :, :])
                ps = p1ps.tile([P, AUG1], f32, tag="ps")
                for k in range(2):
                    nc.tensor.matmul(ps[:], lhsT=xo[:, k, :],
                                     rhs=w1_sb[:, k, :],
                                     start=(k == 0), stop=(k == 1))
                # row: [h1 0:128 | 1.0 | h2 129:257 | 1.0 | as f32 x2 | ad..]
                row = p1row.tile([P, 272], f16, tag="row")
                rf32 = row[:].bitcast(f32)
                nc.scalar.activation(row[:, 0:P], ps[:, 0:P], AF.Copy)
                nc.scalar.activation(row[:, P + 1:IN + 1],
                                     ps[:, P:IN], AF.Copy)
                nc.vector.tensor_copy(rf32[:, 129:133], ps[:, IN:IN + 4])
                nc.vector.memset(row[:, P:P + 1], 1.0)
                nc.vector.memset(row[:, IN + 1:IN + 2], 1.0)
                nc.sync.dma_start(
                    cc1_in[blk * P:(blk + 1) * P, 0:IN + 10],
                    row[:, 0:IN + 10])
                if blk == PB0 - 1:
                    ag_piece(cc1_in, cc1_p0, 0)
            ag_piece(cc1_in, cc1_p1, 1)

        # ================= edge phases ======================================
        # partial L2 sums (lo sub-phase) parked per block in DRAM (f32 --
        # the exp() weights overflow f16): rows [f 0:256 | sumw]
        part_d = dram.tile([CAP, IN + 1], f32, name="part_d")

        def edge_phase(layer, part=None, post_block=None):
            if layer == 1:
                nhead = 2
                b_sb, out_t, lrelu_out = b1_sb, out1T_sb, True
                as_off, ad_off = 129, 131   # f32 col offsets in table rows
                cin = cc1_in
                c0c, c1c = 0, CPB
                with_self, do_epi = True, True
                # chunk roles are per-block (cfg.dsched): lo/hi tails are
                # dense-computed, the rest gathered
                g_rngs = d_rngs = True   # per-block, built in stage_a
            else:
                nhead = 1
                b_sb, out_t, lrelu_out = b2_sb, h2fT_sb, False
                as_off, ad_off = 129, 130
                cin = cc2_in
                d_rngs = []
                SPC = min(CPL, 8)   # pass-A chunk count (one gather call)
                if part == 'lo':
                    c0c, c1c = 0, SPC
                    with_self, do_epi = False, False
                    g_rngs = [(0, SPC, cc2_p0)]
                else:
                    c0c, c1c = SPC, CPB
                    with_self, do_epi = True, True
                    g_rngs = [(SPC, CPL, cc2_p0), (CPL, CPB, cc2_p1)]
            NCF = c1c - c0c
            sfx = f"{layer}{part or ''}"

            estack = ExitStack()
            pi = estack.enter_context(
                tc.tile_pool(name=f"idx{sfx}", bufs=4))
            pso = estack.enter_context(
                tc.tile_pool(name=f"soh{sfx}", bufs=4))
            pg = estack.enter_context(
                tc.tile_pool(name=f"gath{sfx}", bufs=4))
            pw = estack.enter_context(
                tc.tile_pool(name=f"win{sfx}", bufs=4))
            px = (estack.enter_context(tc.tile_pool(name="dsx", bufs=2))
                  if d_rngs else None)
            pps = (estack.enter_context(
                tc.tile_pool(name="dsps", bufs=2, space="PSUM"))
                if d_rngs else None)
            pad_ = estack.enter_context(
                tc.tile_pool(name=f"adps{sfx}", bufs=2, space="PSUM"))
            pe_ = estack.enter_context(tc.tile_pool(name=f"ew{sfx}", bufs=4))
            pm = estack.enter_context(tc.tile_pool(name=f"sw{sfx}", bufs=2))
            pp = estack.enter_context(
                tc.tile_pool(name=f"bps{sfx}", bufs=2, space="PSUM"))
            pt = (estack.enter_context(
                tc.tile_pool(name=f"tps{sfx}", bufs=1, space="PSUM"))
                if do_epi else None)
            po = estack.enter_context(tc.tile_pool(name=f"epi{sfx}", bufs=2))
            ph = (estack.enter_context(
                tc.tile_pool(name=f"h2ps{sfx}", bufs=1, space="PSUM"))
                if layer == 1 else None)

            NSW = NCF + (1 if with_self else 0)
            state = {}

            def stage_a(blk):
                cb8 = blk * CPB * 8
                cbp = blk * CPB * P
                isdt = pi.tile([P, NCF * 8], i16, tag="isd")
                nc.sync.dma_start(isdt[:], isd[:, cb8 + c0c * 8:
                                                cb8 + c1c * 8])
                st = pso.tile([P, NCF * P], bf16, tag="st")
                nc.sync.dma_start(st[:], std[:, cbp + c0c * P:
                                             cbp + c1c * P])
                sall = pso.tile([P, NCF * P], bf16, tag="sall")
                nc.sync.dma_start(sall[:], sald[:, cbp + c0c * P:
                                                cbp + c1c * P])
                win = pw.tile([P, ROWW], f16, tag="win")
                nc.sync.dma_start(win[:], cin[blk * P:(blk + 1) * P, :])
                winf = win[:].bitcast(f32)

                gt = pg.tile([P, NCF, ROWW], f16, tag="gt")
                gtf = gt[:].bitcast(f32)
                if layer == 1:
                    dlo, dhi = cfg.dsched(blk)
                    g_rb = [(0, CPL - dlo, cc1_p0), (CPL, CPB - dhi, cc1_p1)]
                    d_rb = [(CPL - dlo, CPL, 0), (CPB - dhi, CPB, dlo)]
                else:
                    g_rb, d_rb = g_rngs, []
                # gather chunks (chunk j maps to local slot j - c0c)
                MXC = 8
                for r0, r1, tab in g_rb:
                    if r1 <= r0:
                        continue
                    for c0 in range(r0, r1, MXC):
                        c1 = min(c0 + MXC, r1)
                        nc.gpsimd.dma_gather(
                            gt[:, c0 - c0c:c1 - c0c, :], tab[:, :],
                            isdt[:, (c0 - c0c) * 8:(c1 - c0c) * 8],
                            (c1 - c0) * P, (c1 - c0) * P, ROWW)
                # dense chunks (layer 1 only)
                if d_rb:
                    dct_b = sum(cfg.dsched(blk))
                    xt = px.tile([P, DCTMAX, 2, P], f16, tag="xt")
                    xo = XOFF[blk]
                    nc.sync.dma_start(xt[:, 0:dct_b, :, :],
                                      xsTi[:, xo:xo + dct_b, :, :])
                    for r0, r1, xb in d_rb:
                        for j in range(r0, r1):
                            ps = pps.tile([P, AUG1], f32, tag="ps")
                            xi = xb + (j - r0)
                            for k in range(2):
                                nc.tensor.matmul(ps[:], lhsT=xt[:, xi, k, :],
                                                 rhs=w1_sb[:, k, :],
                                                 start=(k == 0), stop=(k == 1))
                            nc.scalar.activation(gt[:, j, 0:P], ps[:, 0:P],
                                                 AF.Copy)
                            nc.scalar.activation(gt[:, j, P + 1:IN + 1],
                                                 ps[:, P:IN], AF.Copy)
                            nc.vector.tensor_copy(gtf[:, j, 129:131],
                                                  ps[:, IN:IN + 2])
                        nc.vector.memset(gt[:, r0:r1, P:P + 1], 1.0)
                        nc.vector.memset(gt[:, r0:r1, IN + 1:IN + 2], 1.0)

                # a_d per edge: ad_ps[e, (j,h)] = st_j^T @ adwin
                adw16 = pe_.tile([P, nhead], f16, tag="adw16")
                nc.vector.tensor_copy(adw16[:],
                                      winf[:, ad_off:ad_off + nhead])
                ad_ps = pad_.tile([P, NCF * nhead], f32, tag="adps")
                for j in range(NCF):
                    nc.tensor.matmul(ad_ps[:, j * nhead:(j + 1) * nhead],
                                     lhsT=st[:, j * P:(j + 1) * P],
                                     rhs=adw16[:], start=(j == 0),
                                     stop=(j == NCF - 1))
                pl = None
                if layer == 2 and do_epi:
                    pl = po.tile([P, IN + 1], f32, tag="pl")
                    nc.sync.dma_start(pl[:],
                                      part_d[blk * P:(blk + 1) * P, :])
                state[blk] = (gt, gtf, win, winf, sall, ad_ps, pl)

            def stage_b(blk):
                gt, gtf, win, winf, sall, ad_ps, pl = state.pop(blk)
                # edge weights w = exp(lrelu(a_s + a_d, .2))
                ew = pe_.tile([P, NCF * nhead], f32, tag="ew")
                wv = pe_.tile([P, NCF * nhead], bf16, tag="wv")
                as_v = gtf[:, :, as_off:as_off + nhead]
                nc.vector.tensor_tensor(ew[:], as_v, ad_ps[:], op=ALU.add)
                ewl = pe_.tile([P, NCF * nhead], f32, tag="ewl")
                nc.vector.scalar_tensor_tensor(
                    out=ewl[:], in0=ew[:], scalar=0.2, in1=ew[:],
                    op0=ALU.mult, op1=ALU.max)
                nc.scalar.activation(wv[:], ewl[:], AF.Exp)

                if with_self:
                    # self-loop weights from the window rows
                    ws = pe_.tile([P, nhead], f32, tag="ws")
                    nc.vector.tensor_tensor(
                        ws[:], winf[:, as_off:as_off + nhead],
                        winf[:, ad_off:ad_off + nhead], op=ALU.add)
                    wt = pe_.tile([P, nhead], f32, tag="wt")
                    nc.vector.scalar_tensor_tensor(
                        out=wt[:], in0=ws[:], scalar=0.2, in1=ws[:],
                        op0=ALU.mult, op1=ALU.max)
                    nc.scalar.activation(wt[:], wt[:], AF.Exp)

                # weighted one-hot scatter: sw_h = sall * w_h (stride-0 bcast)
                swh = []
                for h in range(nhead):
                    wvh = wv[:, h:h + 1]
                    wview = AP(wvh.tensor, wvh.offset,
                               [wvh.ap[0], [nhead, NCF], [0, P]])
                    swt = pm.tile([P, NCF * P], bf16, tag=f"swh{h}")
                    nc.vector.tensor_tensor(swt[:], sall[:], wview,
                                            op=ALU.mult)
                    swh.append(swt)

                # single psum accumulation group across all j/h matmuls:
                # start only on the first, stop only on the very last.  Table
                # rows carry a literal 1.0 after each head's features, so one
                # matmul accumulates both the weighted feature sum and the
                # softmax denominator:
                # bp layout [f1 0:128 | sumw1 | f2 129:257 | sumw2] (2 heads)
                # or [f 0:256 | sumw] (1 head).
                HB = P + 1 if nhead == 2 else IN + 1
                bp = pp.tile([P, nhead * HB], f32, tag="bp")
                for j in range(NSW):
                    last = (j == NSW - 1)
                    selfc = with_self and (j == NCF)
                    for h in range(nhead):
                        if selfc:
                            sw = pm.tile([P, P], bf16, tag="sw")
                            nc.vector.tensor_scalar(
                                out=sw[:], in0=identb_sb[:],
                                scalar1=wt[:, h:h + 1],
                                scalar2=None, op0=ALU.mult)
                            lhsT = sw[:]
                        else:
                            lhsT = swh[h][:, j * P:(j + 1) * P]
                        c0, c1 = h * HB, (h + 1) * HB
                        rhs = win[:, c0:c1] if selfc else gt[:, j, c0:c1]
                        nc.tensor.matmul(bp[:, c0:c1], lhsT=lhsT,
                                         rhs=rhs,
                                         start=(j == 0 and h == 0),
                                         stop=(last and h == nhead - 1))

                if not do_epi:
                    # L2 lo sub-phase: park the partial sums, finish later
                    pf = po.tile([P, IN + 1], f32, tag="pf")
                    nc.scalar.activation(pf[:], bp[:], AF.Copy)
                    nc.sync.dma_start(part_d[blk * P:(blk + 1) * P, :],
                                      pf[:])
                    if post_block is not None:
                        post_block(blk)
                    return

                # ---- block epilogue
                if layer == 2:
                    tb = po.tile([P, IN + 1], f32, tag="tb")
                    nc.vector.tensor_tensor(tb[:], bp[:], pl[:],
                                            op=ALU.add)
                    bpv = tb
                else:
                    bpv = bp
                rec = po.tile([P, nhead], f32, tag="rec")
                for h in range(nhead):
                    nc.vector.reciprocal(rec[:, h:h + 1],
                                         bpv[:, (h + 1) * HB - 1:
                                             (h + 1) * HB])
                ti = po.tile([P, IN], f32, tag="ti")
                if nhead == 2:
                    nc.scalar.activation(ti[:, 0:P], bpv[:, 0:P], AF.Copy,
                                         scale=rec[:, 0:1])
                    nc.scalar.activation(ti[:, P:IN], bpv[:, HB:HB + P],
                                         AF.Copy, scale=rec[:, 1:2])
                else:
                    nc.scalar.activation(ti[:], bpv[:, 0:IN], AF.Copy,
                                         scale=rec[:, 0:1])
                nc.vector.tensor_tensor(ti[:], ti[:], b_sb[:], op=ALU.add)
                if lrelu_out:
                    # lrelu(x) = 0.01x + relu(0.99x)
                    tr = po.tile([P, IN], f32, tag="tr")
                    nc.scalar.activation(tr[:], ti[:], AF.Relu, scale=0.99)
                    nc.vector.scalar_tensor_tensor(
                        out=ti[:], in0=ti[:], scalar=0.01, in1=tr[:],
                        op0=ALU.mult, op1=ALU.add)
                for k in range(2):
                    tp = pt.tile([P, P], f32, tag="tp")
                    nc.tensor.transpose(tp[:], ti[:, k * P:(k + 1) * P],
                                        ident_sb[:])
                    nc.scalar.activation(out_t[:, k, blk * P:(blk + 1) * P],
                                         tp[:], AF.Copy)

                if layer == 1:
                    hp = ph.tile([P, AUG2], f32, tag="hp")
                    for k in range(2):
                        nc.tensor.matmul(
                            hp[:],
                            lhsT=out1T_sb[:, k, blk * P:(blk + 1) * P],
                            rhs=w2_sb[:, k, :], start=(k == 0), stop=(k == 1))
                    # L2 row: [h 0:256 | 1.0 | a_s f32 @129 | a_d f32 @130]
                    row2 = po.tile([P, ROWW], f16, tag="row2")
                    nc.scalar.activation(row2[:, 0:OUT], hp[:, 0:OUT], AF.Copy)
                    nc.vector.memset(row2[:, OUT:OUT + 2], 1.0)
                    r2f = row2[:].bitcast(f32)
                    nc.vector.tensor_copy(r2f[:, 129:131],
                                          hp[:, OUT:OUT + 2])
                    nc.sync.dma_start(
                        cc2_in[blk * P:(blk + 1) * P, 0:OUT + 6],
                        row2[:, 0:OUT + 6])

                if post_block is not None:
                    post_block(blk)

            for b in range(NBLK + 2):
                if b >= 2:
                    stage_b(b - 2)
                if b < NBLK:
                    stage_a(b)

            estack.close()

        def _post1(blk):
            if blk == PB0 - 1:
                ag_piece(cc2_in, cc2_p0, 0)
            elif blk == NBLK - 1:
                ag_piece(cc2_in, cc2_p1, 1)

        edge_phase(1, post_block=_post1)
        edge_phase(2, part='lo')

        # ====== P5 head, interleaved into edge phase 2 ======================
        hstack = ExitStack()
        hps = hstack.enter_context(
            tc.tile_pool(name="hps", bufs=1, space="PSUM"))
        hsb = hstack.enter_context(tc.tile_pool(name="hsb", bufs=2))
        sps = hstack.enter_context(
            tc.tile_pool(name="sps", bufs=1, space="PSUM"))
        hepi = hstack.enter_context(tc.tile_pool(name="hepi", bufs=2))

        NTL = []
        _st = 0
        while _st < CAP:
            _w = min(512, CAP - _st)
            NTL.append((_st, _w))
            _st += _w

        def head_slice(st, wdt):
            nump = sps.tile([KH, 512], f32, tag="nump")
            nrmp = sps.tile([KH, 512], f32, tag="nrmp")
            # numerator: num[k,n] = sum_f Gmu[f,k] * h2[f,n]
            for f in range(2):
                nc.tensor.matmul(nump[:, 0:wdt],
                                 lhsT=gmu_sb[:, f, :],
                                 rhs=h2fT_sb[:, f, st:st + wdt],
                                 start=(f == 0), stop=(f == 1))
            for k in range(KH):
                hp = hps.tile([P, 512], f32, tag="hp")
                for f in range(2):
                    nc.tensor.matmul(hp[:, 0:wdt],
                                     lhsT=g_sb[:, f, k * P:(k + 1) * P],
                                     rhs=h2fT_sb[:, f, st:st + wdt],
                                     start=(f == 0), stop=(f == 1))
                sq16 = hsb.tile([P, 512], f16, tag="sq16")
                nc.scalar.activation(sq16[:, 0:wdt], hp[:, 0:wdt], AF.Square)
                nc.tensor.matmul(nrmp[:, 0:wdt],
                                 lhsT=on_sb[:, k * KH:(k + 1) * KH],
                                 rhs=sq16[:, 0:wdt], start=(k == 0),
                                 stop=(k == KH - 1))
            sq = hepi.tile([KH, 512], f32, tag="sqr")
            # sqrt(x) = exp(0.5*ln(x)) -- keeps ACT on the ln/exp table set
            nc.scalar.activation(sq[:, 0:wdt], nrmp[:, 0:wdt], AF.Ln)
            nc.scalar.activation(sq[:, 0:wdt], sq[:, 0:wdt], AF.Exp,
                                 scale=0.5)
            nc.vector.tensor_scalar(out=sq[:, 0:wdt], in0=sq[:, 0:wdt],
                                    scalar1=cmu_sb[:, 0:1], scalar2=1e-8,
                                    op0=ALU.mult, op1=ALU.max)
            nc.vector.reciprocal(sq[:, 0:wdt], sq[:, 0:wdt])
            res = hepi.tile([KH, 512], f32, tag="res")
            nc.vector.tensor_tensor(res[:, 0:wdt], nump[:, 0:wdt],
                                    sq[:, 0:wdt], op=ALU.mult)
            nc.sync.dma_start(outT[:, st:st + wdt], res[:, 0:wdt])

        _emitted = [0]

        def _post2(blk):
            done = (blk + 1) * P
            while _emitted[0] < len(NTL):
                st, wdt = NTL[_emitted[0]]
                if st + wdt > done:
                    break
                head_slice(st, wdt)
                _emitted[0] += 1

        edge_phase(2, part='hi', post_block=_post2)
        while _emitted[0] < len(NTL):
            st, wdt = NTL[_emitted[0]]
            head_slice(st, wdt)
            _emitted[0] += 1
        hstack.close()

    nc.compile()
    return nc


# ======================= host-side preparation ==============================

def _wrap16(flat):
    """idx flat [n] -> wrapped int16 [128, n//16]; pos i -> (i%16, i//16),
    replicated across the 8 Q7-core stripes."""
    n = len(flat)
    out = np.zeros((P, n // 16), np.int16)
    cols = np.arange(n) // 16
    rows = np.arange(n) % 16
    for r in range(8):
        out[r * 16 + rows, cols] = flat
    return out


def _balance_bins(deg, nbins, cap):
    """Greedy multiway partition: assign nodes to bins balancing total degree,
    each bin holding at most `cap` nodes.  Returns bin id per node."""
    import heapq
    n = len(deg)
    order = np.argsort(-deg, kind="stable")
    binid = np.empty(n, np.int32)
    counts = np.zeros(nbins, np.int32)
    heap = [(0, b) for b in range(nbins)]
    heapq.heapify(heap)
    for nd in order:
        while True:
            load, b = heapq.heappop(heap)
            if counts[b] < cap:
                break
        binid[nd] = b
        counts[b] += 1
        if counts[b] < cap:
            heapq.heappush(heap, (load + int(deg[nd]), b))
    return binid


def prep_host(x, edge_index, W1, a_src1, a_dst1, b1, W2, a_src2, a_dst2, b2,
              g, mu, world=8):
    import ml_dtypes
    x16 = np.asarray(x, np.float32).astype(np.float16)
    N = x16.shape[0]
    NBLK = int(np.ceil(N / world / P))
    CAP = NBLK * P
    nbins = world * NBLK
    PB0 = min(NBLK, HALF // (world * P))
    PB1 = NBLK - PB0

    src = np.asarray(edge_index[0]).astype(np.int64)
    dst = np.asarray(edge_index[1]).astype(np.int64)

    # --- balanced global node -> (core, block, slot) assignment
    deg = np.bincount(dst, minlength=N)
    binid = _balance_bins(deg, nbins, P)
    order = np.lexsort((np.arange(N), binid))
    # local slot position within the core's shard
    lpos = np.empty(N, np.int64)
    nxt = np.arange(nbins, dtype=np.int64) * P
    for nd in order:
        b = binid[nd]
        lpos[nd] = nxt[b]
        nxt[b] += 1
    node_core = binid // NBLK
    node_blk = binid % NBLK
    lpos -= node_core * CAP              # position within own core [0, CAP)

    # global cc table position: AllGather piece-major
    # piece0 rows: [core, blocks 0:PB0]; piece1: [core, blocks PB0:NBLK]
    in_p1 = node_blk >= PB0
    gpos = np.where(
        ~in_p1,
        node_core * (PB0 * P) + lpos,
        world * PB0 * P + node_core * (PB1 * P) + (lpos - PB0 * P))

    # per-core list of node ids in shard slot order (-1 = empty slot)
    idxmaps = []
    for c in range(world):
        m = np.full(CAP, -1, np.int64)
        mask = node_core == c
        m[lpos[mask]] = np.nonzero(mask)[0]
        idxmaps.append(m)

    # --- edges grouped by (core, block) of dst
    ecore = node_core[dst]
    eblk = node_blk[dst]
    gkey = ecore * NBLK + eblk
    gorder = np.argsort(gkey, kind="stable")
    srcg, dstg, gkeyg = src[gorder], dst[gorder], gkey[gorder]
    starts = np.concatenate(
        [[0], np.cumsum(np.bincount(gkeyg, minlength=nbins))])

    ed = {}
    CPL = CPH = 1
    for c in range(world):
        for b in range(NBLK):
            gid = c * NBLK + b
            es = srcg[starts[gid]:starts[gid + 1]]
            eds = dstg[starts[gid]:starts[gid + 1]]
            dloc = (lpos[eds] - b * P).astype(np.int64)
            tl = gpos[es]
            lo = tl < world * PB0 * P
            ed[(c, b)] = (es, tl, lo, dloc)
            CPL = max(CPL, int(np.ceil(lo.sum() / P)))
            CPH = max(CPH, int(np.ceil((~lo).sum() / P)))

    cfg = CFG(N=N, W=world, NBLK=NBLK, CPL=CPL, CPH=CPH, idxmaps=idxmaps)
    CPB = cfg.CPB
    XOFF = cfg.xoffs
    ar128 = np.arange(P, dtype=np.int64)

    def build_core(c):
        isd = np.zeros((P, NBLK * CPB * 8), np.int16)
        sth = np.zeros((P, NBLK * CPB * P), ml_dtypes.bfloat16)
        salh = np.zeros((P, NBLK * CPB * P), ml_dtypes.bfloat16)
        srcs = np.zeros(XOFF[NBLK] * P, np.int64)     # dense-chunk x rows
        for b in range(NBLK):
            es, tl, lo, dloc = ed[(c, b)]
            fl = np.zeros(CPB * P, np.int64)      # slot -> table idx (pad 0)
            fd = np.full(CPB * P, -1, np.int64)   # slot -> dst_local (pad -1)
            fs = np.zeros(CPB * P, np.int64)      # slot -> src node id
            ilo = np.where(lo)[0]
            ihi = np.where(~lo)[0]
            fl[:len(ilo)] = tl[ilo]
            fd[:len(ilo)] = dloc[ilo]
            fs[:len(ilo)] = es[ilo]
            fl[CPL * P:CPL * P + len(ihi)] = tl[ihi] - world * PB0 * P
            fd[CPL * P:CPL * P + len(ihi)] = dloc[ihi]
            fs[CPL * P:CPL * P + len(ihi)] = es[ihi]
            cb8 = b * CPB * 8
            isd[:, cb8:cb8 + CPB * 8] = _wrap16(fl)
            # one-hots from fd [CPB, P]
            fdm = fd.reshape(CPB, P)
            oh = (fdm[:, :, None] == ar128)                 # [j, e, d]
            cbp = b * CPB * P
            sth[:, cbp:cbp + CPB * P] = \
                oh.transpose(2, 0, 1).reshape(P, CPB * P)   # st[d,(j,e)]
            salh[:, cbp:cbp + CPB * P] = \
                oh.transpose(1, 0, 2).reshape(P, CPB * P)   # sall[e,(j,d)]
            # dense chunk sources: lo [CPL-dlo:CPL], hi [CPB-dhi:CPB]
            fsm = fs.reshape(CPB, P)
            dlo, dhi = cfg.dsched(b)
            o = XOFF[b] * P
            if dlo:
                srcs[o:o + dlo * P] = fsm[CPL - dlo:CPL].ravel()
            if dhi:
                srcs[o + dlo * P:o + (dlo + dhi) * P] = \
                    fsm[CPB - dhi:CPB].ravel()
        xs = x16[srcs]                            # [XOFF[-1]*P, IN]
        xsT = np.ascontiguousarray(
            xs.reshape(XOFF[NBLK], P, 2, P).transpose(3, 0, 2, 1))
        return isd, sth, salh, xsT

    # weights
    W1 = np.asarray(W1, np.float32)
    W2 = np.asarray(W2, np.float32)
    W1r = W1.reshape(H1, MD, IN)
    Ps1 = np.einsum("hdi,hd->ih", W1r, np.asarray(a_src1, np.float32))
    Pd1 = np.einsum("hdi,hd->ih", W1r, np.asarray(a_dst1, np.float32))
    W1aug = np.concatenate([W1.T, Ps1, Pd1], axis=1)
    Ps2 = W2.T @ np.asarray(a_src2, np.float32)[0][:, None]
    Pd2 = W2.T @ np.asarray(a_dst2, np.float32)[0][:, None]
    W2aug = np.concatenate([W2.T, Ps2, Pd2], axis=1)
    AUG1, AUG2 = IN + 4, IN + 2
    w1s = W1aug.reshape(2, P, AUG1).transpose(1, 0, 2).astype(np.float16)
    w2s = W2aug.reshape(2, P, AUG2).transpose(1, 0, 2).astype(np.float16)

    gm = np.asarray(g, np.float32)
    gsd = gm.reshape(2, P, KH * P).transpose(1, 0, 2).astype(np.float16)
    mu = np.asarray(mu, np.float32)
    # Gmu[f, k] = sum_m g[f, k*MD+m] * mu[k, m]
    gmu = np.einsum("fkm,km->fk", gm.reshape(IN, KH, MD), mu)
    gmud = gmu.reshape(2, P, KH).transpose(1, 0, 2).astype(np.float16)
    onesd = np.zeros((P, KH * KH), np.float16)
    for k in range(KH):
        onesd[:, k * KH + k] = 1.0
    cmu = np.linalg.norm(mu, axis=1)[:, None].astype(np.float32)
    b1b = np.broadcast_to(np.asarray(b1, np.float32), (P, HID)).copy()
    b2b = np.broadcast_to(np.asarray(b2, np.float32), (P, OUT)).copy()
    ident = np.eye(P, dtype=np.float32)
    identb = np.eye(P, dtype=ml_dtypes.bfloat16)

    shared = dict(w1s=w1s, w2s=w2s, gs=gsd, gmu=gmud, onesd=onesd, cmu=cmu,
                  b1b=b1b, b2b=b2b, ident=ident, identb=identb)
    in_maps = []
    for c in range(world):
        m = idxmaps[c]
        own = np.where(m >= 0, m, 0)
        xo = x16[own]
        xo[m < 0] = 0
        xoT = np.ascontiguousarray(
            xo.reshape(NBLK, P, 2, P).transpose(3, 0, 2, 1))
        isd_c, st_c, sal_c, xsT_c = build_core(c)
        mm = dict(shared)
        mm.update(xoTi=xoT, xsTi=xsT_c, isd=isd_c, std=st_c, sald=sal_c)
        in_maps.append(mm)
    return cfg, in_maps


def assemble(cfg, outs):
    N = cfg.N
    full = np.zeros((N, KH), np.float32)
    for c in range(cfg.W):
        o = np.asarray(outs[c]["outT"])      # [KH, SHARD_CAP]
        m = cfg.idxmaps[c]
        valid = m >= 0
        full[m[valid], :] = o[:, valid].T
    return full


_CACHE = {}


def kernel(**inputs):
    world = 8
    cfg, in_maps = prep_host(world=world, **inputs)
    key = (cfg.N, cfg.W, cfg.CPL, cfg.CPH)
    if key not in _CACHE:
        _CACHE[key] = build_program(cfg)
    nc = _CACHE[key]

    from concourse.bass_utils import run_bass_kernel_spmd
    res = run_bass_kernel_spmd(nc, in_maps, core_ids=list(range(world)))
    return assemble(cfg, res.results)
